# revision 2
# baseline (speedup 1.0000x reference)
"""MixHop GNN v2: source-stationary SpMM + ReduceScatter on 8 trn2 cores.

vs v1 (gather-from-replicated-table + AllGather):
 - Each core owns src shard [12544 rows]; every SpMM gathers ONLY from the
   local-shard table (int16 idx, no quartering) and produces a partial for
   ALL 784 dst tiles; a ReduceScatter(add) returns the local shard of the
   aggregate. RS is ~2-3x cheaper than AG in the cost model and is split
   into 2 phase-halves that overlap with compute.
 - Edge slots: per rank-of-98-tiles: 24 quads (4 tiles) + 1 pair; each tile
   has one 128-slot c1 chunk; each group has ONE shared 128-slot overflow
   chunk (c2). 125952 slots vs 150528 (v1) for the same edges.
 - dinv[dst] scaling folded into the PSUM drains (pre-RS; commutes with the
   sum), so RS outputs are directly h-ready or table-ready (dinv^2).
 - L2/L3 first hops share one pass: table [12544, 256] = [dinv*t1|dinv*t2],
   512B gather elements (no <512B DMA penalty), one sel serves both.
 - Transposes via HWDGE dma_start_transpose instead of PE+copy.
"""
import os
import numpy as np
import ml_dtypes

import concourse.bass as bass
import concourse.mybir as mybir
import concourse.tile as tile
from concourse import library_config
from concourse.bass_utils import run_bass_kernel_spmd

bf16 = mybir.dt.bfloat16
f32 = mybir.dt.float32
i16 = mybir.dt.int16
BF = ml_dtypes.bfloat16

N = 100000
NC = 8
P = 128
D = 128
SHARD = 12544
NPAD = SHARD * NC
RT = 98                  # tiles per rank
NQ = 24                  # quads per rank (plus 1 trailing pair)
PH1T = 48                # tiles in phase 1 (quads 0..11)
PH2T = 50                # tiles in phase 2 (quads 12..23 + pair)
GSZ = 512
NG = (SHARD + GSZ - 1) // GSZ   # 25 transform groups
EPS = 1e-5

# groups per rank: 24 quads + 1 pair; chunks: quads 5, pair 3
CPR = NQ * 5 + 3         # 123 chunks per rank
NSLOT = NC * CPR * P     # 125952
NC1 = 784                # c1 chunks total
NC2 = NC * (NQ + 1)      # 200 overflow chunks

AOP = mybir.AluOpType
AF = mybir.ActivationFunctionType

_SKIP_WAITSPLIT = (mybir.InstEventSemaphore,)


def _split_excess_waits(nc, keep=1):
    n = 0
    uid = [0]
    for fn in nc.m.functions:
        for blk in fn.blocks:
            insts = list(blk.instructions)
            out = []
            for inst in insts:
                si = inst.sync_info
                if (si is not None and si.on_wait and len(si.on_wait) > keep
                        and not isinstance(inst, _SKIP_WAITSPLIT)):
                    waits = list(si.on_wait)
                    extra, rest = waits[:-keep], waits[-keep:]
                    for w in extra:
                        uid[0] += 1
                        out.append(mybir.InstEventSemaphore(
                            name=f"evws_{uid[0]}",
                            engine=inst.engine,
                            ins=[], outs=[],
                            sync_info=mybir.SyncInfo(on_wait=[w], on_update=[]),
                        ))
                        n += 1
                    inst.sync_info = mybir.SyncInfo(
                        on_wait=rest, on_update=list(si.on_update or []))
                out.append(inst)
            if len(out) != len(insts):
                blk.instructions = out
    return n


# ------------------------------------------------------------------ batches
def batch_plan():
    """Static per-(phase, rank) batch structure.

    Returns list over phases of list over ranks of batches; each batch is a
    list of groups; each group is (kind, tiles) with kind in {"q","p"} and
    tiles = local tile indices (lt) in ascending order.
    """
    phases = []
    for ph in (0, 1):
        ranks = []
        for r in range(NC):
            if ph == 0:
                quads = [("q", [4 * q + i for i in range(4)])
                         for q in range(12)]
                batches = [quads[:6], quads[6:]]
            else:
                quads = [("q", [4 * q + i for i in range(4)])
                         for q in range(12, 24)]
                pair = ("p", [96, 97])
                batches = [quads[:6], quads[6:] + [pair]]
            ranks.append(batches)
        phases.append(ranks)
    return phases


# ---------------------------------------------------------------- host prep
def _host_prep(x, edge_index):
    row = edge_index[0].astype(np.int64)
    col = edge_index[1].astype(np.int64)
    deg = np.bincount(col, minlength=N).astype(np.float64)
    dinv = np.where(deg > 0, 1.0 / np.sqrt(np.maximum(deg, 1.0)), 0.0)
    s1 = dinv * np.bincount(col, weights=dinv[row], minlength=N)
    s2 = dinv * np.bincount(col, weights=dinv[row] * s1[row], minlength=N)

    dinv_pad = np.zeros(NPAD)
    dinv_pad[:N] = dinv
    x_pad = np.zeros((NPAD, D), np.float32)
    x_pad[:N] = x
    s1_pad = np.zeros(NPAD, np.float32)
    s1_pad[:N] = s1
    s2_pad = np.zeros(NPAD, np.float32)
    s2_pad[:N] = s2
    ones_pad = np.zeros(NPAD, np.float32)
    ones_pad[:N] = 1.0

    plan = batch_plan()
    cores = []
    for c in range(NC):
        lo = c * SHARD
        m = (row >= lo) & (row < lo + SHARD)
        r_l = (row[m] - lo).astype(np.int64)
        cc = col[m]
        t = cc >> 7
        order = np.argsort(t, kind="stable")
        r_l, cc, t = r_l[order], cc[order], t[order]
        cnt = np.bincount(t, minlength=784)
        starts = np.zeros(785, np.int64)
        starts[1:] = np.cumsum(cnt)

        idx = np.zeros(NSLOT, np.int16)
        ld1 = np.full((P, NC1), -1.0, np.float32)
        ld2a = np.full((P, NC2), -1.0, np.float32)
        ld2b = np.full((P, NC2), -1.0, np.float32)
        slot = 0
        c1i = 0
        c2i = 0
        for ph in (0, 1):
            for r in range(NC):
                for batch in plan[ph][r]:
                    for kind, lts in batch:
                        ov_idx = []
                        ov_rel = []
                        for j, lt in enumerate(lts):
                            gt = r * RT + lt
                            s, e = starts[gt], starts[gt + 1]
                            n1 = min(e - s, P)
                            idx[slot:slot + n1] = r_l[s:s + n1]
                            ld1[:n1, c1i] = (cc[s:s + n1] & 127)
                            slot += P
                            c1i += 1
                            if e - s > n1:
                                ov_idx.append(r_l[s + n1:e])
                                ov_rel.append((cc[s + n1:e] & 127) + P * j)
                        ov_idx = (np.concatenate(ov_idx) if ov_idx
                                  else np.zeros(0, np.int64))
                        ov_rel = (np.concatenate(ov_rel) if ov_rel
                                  else np.zeros(0, np.int64))
                        no = len(ov_idx)
                        assert no <= P, f"overflow {no} > 128"
                        idx[slot:slot + no] = ov_idx
                        ra = ov_rel.astype(np.float32)
                        ld2a[:no, c2i] = np.where(ov_rel < 256, ra, -1.0)
                        ld2b[:no, c2i] = np.where(ov_rel >= 256, ra - 256, -1.0)
                        slot += P
                        c2i += 1
        assert slot == NSLOT and c1i == NC1 and c2i == NC2

        # idx16: per batch segment, 16-wrap + 8x replicate
        idx16 = np.zeros((P, NSLOT // 16), np.int16)
        pos = 0
        for ph in (0, 1):
            for r in range(NC):
                for batch in plan[ph][r]:
                    nch = sum(len(g[1]) + 1 for g in batch)
                    nidx = nch * P
                    seg = idx[pos:pos + nidx]
                    blk = seg.reshape(-1, 16).T
                    idx16[:, pos // 16:(pos + nidx) // 16] = np.tile(blk, (8, 1))
                    pos += nidx

        dl = dinv_pad[lo:lo + SHARD]
        xtbl = (dl[:, None] * x_pad[lo:lo + SHARD]).astype(BF)
        xt = x_pad[lo:lo + SHARD].T.astype(BF)
        dloc1 = dl.reshape(RT, P).T.astype(np.float32)
        dloc2 = (dl ** 2).reshape(RT, P).T.astype(np.float32)
        dlI = np.where(dl > 0, 1.0 / np.maximum(dl, 1e-30), 0.0)
        dlocI = dlI.reshape(RT, P).T.astype(np.float32)
        dg1 = dinv_pad.reshape(784, P).T.astype(np.float32)
        dg2 = (dinv_pad ** 2).reshape(784, P).T.astype(np.float32)
        dgAB = np.empty((P, 784 * 2), np.float32)
        dgAB[:, 0::2] = dg1
        dgAB[:, 1::2] = dg2
        srow = np.concatenate([s1_pad[lo:lo + SHARD], s2_pad[lo:lo + SHARD]])
        onesr = ones_pad[lo:lo + SHARD]
        cores.append(dict(idx16=idx16, ld1=ld1, ld2a=ld2a, ld2b=ld2b,
                          xtbl=xtbl, xt=xt, dloc1=dloc1, dloc2=dloc2,
                          dlocI=dlocI, dg1=dg1, dgAB=dgAB,
                          srow=srow[None, :].astype(BF),
                          onesr=onesr[None, :].astype(BF)))
    return cores


def _pack_consts(core, W0, b0, W1, b1, W2, b2, bn_g, bn_b):
    iota1 = np.tile(np.arange(P, dtype=np.float32), (P, 1)).astype(BF)
    iota2 = np.tile(np.arange(2 * P, dtype=np.float32), (P, 1)).astype(BF)
    ident = np.eye(P, dtype=np.float32)
    w0c = np.concatenate([W0[j] for j in range(3)], axis=1)
    blocks = []
    for W in (W1, W2):
        for j in range(3):
            for b in range(3):
                blocks.append(W[j][b * P:(b + 1) * P, :])
    w12c = np.concatenate(blocks, axis=1)
    cb = np.concatenate([iota1, iota2, core["ld1"].astype(BF),
                         core["ld2a"].astype(BF), core["ld2b"].astype(BF),
                         ident.astype(BF), w0c.astype(BF), w12c.astype(BF)],
                        axis=1)
    br = np.concatenate([b0.reshape(1, -1), b1.reshape(1, -1),
                         b2.reshape(1, -1)], axis=1)
    cr = np.concatenate([br.astype(np.float32),
                         core["srow"].astype(np.float32),
                         core["onesr"].astype(np.float32)], axis=1).astype(BF)
    bng = np.stack([bn_g[l].reshape(3, P).T for l in range(2)], axis=0)
    bnb = np.stack([bn_b[l].reshape(3, P).T for l in range(2)], axis=0)
    cf = np.concatenate([core["dloc1"], core["dloc2"], core["dlocI"],
                         bng[0], bng[1], bnb[0], bnb[1], ident,
                         core["dg1"], core["dgAB"]], axis=1)
    return cb.astype(BF), cr, cf.astype(np.float32)


NCB = P + 2 * P + NC1 + 2 * NC2 + P + 3 * P + 18 * P
NCR = 9 * P + 2 * SHARD + SHARD
NCF = 3 * RT + 12 + P + 784 * 3


# ---------------------------------------------------------------- device
def _build(phase_sel="full", for_sim=False):
    plan = batch_plan()
    nc = bass.Bass(num_devices=NC)
    xtbl_t = nc.dram_tensor("xtbl", [SHARD, P], bf16, kind="ExternalInput")
    xt_t = nc.dram_tensor("xt", [P, SHARD], bf16, kind="ExternalInput")
    idx_t = nc.dram_tensor("idx", [P, NSLOT // 16], i16, kind="ExternalInput")
    cb_t = nc.dram_tensor("cb", [P, NCB], bf16, kind="ExternalInput")
    cr_t = nc.dram_tensor("cr", [1, NCR], bf16, kind="ExternalInput")
    cf_t = nc.dram_tensor("cf", [P, NCF], f32, kind="ExternalInput")
    out_t = nc.dram_tensor("out", [SHARD, 3 * P], f32, kind="ExternalOutput")

    with tile.TileContext(nc) as tc:
        with (
            tc.tile_pool(name="const", bufs=1) as cpool,
            tc.tile_pool(name="gath", bufs=2) as gpool,
            tc.tile_pool(name="sel", bufs=2) as spool,
            tc.tile_pool(name="stg", bufs=2) as stgpool,
            tc.tile_pool(name="work", bufs=3) as wpool,
            tc.tile_pool(name="hb", bufs=2) as hpool,
            tc.tile_pool(name="stat", bufs=1) as stpool,
            tc.tile_pool(name="psp", bufs=3, space="PSUM") as ps_sp,
            tc.tile_pool(name="ptf", bufs=2, space="PSUM") as ps_tf,
            tc.tile_pool(name="ptp", bufs=2, space="PSUM") as ps_tp,
            tc.tile_pool(name="dram", bufs=1, space="DRAM") as dpool,
        ):
            nc.gpsimd.load_library(library_config.mlp)

            idx_sb = cpool.tile([P, NSLOT // 16], i16)
            nc.sync.dma_start(idx_sb[:], idx_t[:])
            cb_sb = cpool.tile([P, NCB], bf16)
            nc.sync.dma_start(cb_sb[:], cb_t[:])
            bias_sb = cpool.tile([1, 9 * P], bf16)
            nc.sync.dma_start(bias_sb[:], cr_t[:1, :9 * P])
            cf_sb = cpool.tile([P, NCF], f32)
            nc.sync.dma_start(cf_sb[:], cf_t[:])

            o = 0
            iota1_sb = cb_sb[:, o:o + P]; o += P
            iota2_sb = cb_sb[:, o:o + 2 * P]; o += 2 * P
            ld1_sb = cb_sb[:, o:o + NC1]; o += NC1
            ld2a_sb = cb_sb[:, o:o + NC2]; o += NC2
            ld2b_sb = cb_sb[:, o:o + NC2]; o += NC2
            ident_sb = cb_sb[:, o:o + P]; o += P
            w0_sb = cb_sb[:, o:o + 3 * P]; o += 3 * P
            w12_sb = cb_sb[:, o:o + 18 * P]

            def wblk(l, j, b):
                i = ((l - 1) * 9 + j * 3 + b) * P
                return w12_sb[:, i:i + P]

            def brow(l, j):
                i = (l * 3 + j) * P
                return bias_sb[:1, i:i + P]

            def load_row(which, n0, w, tag):
                i = 9 * P + which * SHARD + n0
                rt_ = wpool.tile([1, GSZ], bf16, name="row", tag=tag)
                nc.sync.dma_start(rt_[:1, :w], cr_t[:1, i:i + w])
                return rt_[:1, :w]

            dloc1_sb = cf_sb[:, 0:RT]
            dloc2_sb = cf_sb[:, RT:2 * RT]
            dlocI_sb = cf_sb[:, 2 * RT:3 * RT]

            def bng(l, b):
                i = 3 * RT + l * 3 + b
                return cf_sb[:, i:i + 1]

            def bnb(l, b):
                i = 3 * RT + 6 + l * 3 + b
                return cf_sb[:, i:i + 1]

            identf_sb = cf_sb[:, 3 * RT + 12:3 * RT + 12 + P]
            _o2 = 3 * RT + 12 + P
            dg1_sb = cf_sb[:, _o2:_o2 + 784]
            dgAB_sb = cf_sb[:, _o2 + 784:_o2 + 784 * 3]
            dg2_sb = (dgAB_sb.rearrange("p (t two) -> p t two", two=2)
                      [:, :, 1])

            # DRAM buffers
            def dbuf(name, rows, colsw, dt=bf16):
                return dpool.tile([rows, colsw], dt, name=name)

            # partials: [*,128] for single passes, [*,256] for paired
            partR = [dbuf("pR1", NC * 6144, P), dbuf("pR2", NC * 6400, P)]
            partAB = [dbuf("pAB1", NC * 6144, 2 * P),
                      dbuf("pAB2", NC * 6400, 2 * P)]
            u1buf = dbuf("u1buf", SHARD, P)
            y1buf = dbuf("y1buf", SHARD, P)       # dinv*R1
            y2buf = dbuf("y2buf", SHARD, P)       # dinv*R2
            tbl2 = dbuf("tbl2", SHARD, P)         # dinv^2*R1
            tblpair = [dbuf(f"tblpair{l}", SHARD, 2 * P) for l in (0, 1)]
            rab = [dbuf(f"rab{l}", SHARD, 2 * P) for l in (0, 1)]
            tblu = [dbuf(f"tblu{l}", SHARD, P) for l in (0, 1)]
            rb2 = [dbuf(f"rb2_{l}", SHARD, P) for l in (0, 1)]
            raw1 = [dbuf(f"raw1_{b}", P, SHARD) for b in range(3)]
            raw20 = dbuf("raw2_0", P, SHARD)
            arin = [dpool.tile([P, 8], f32, name=f"arin{l}") for l in range(2)]
            arout = [dpool.tile([P, 8], f32, name=f"arout{l}",
                                addr_space="Shared") for l in range(2)]

            nreg1 = nc.gpsimd.to_reg(30 * P)
            nreg2 = nc.gpsimd.to_reg(33 * P)

            # ------------------------------------------------ spmm pass
            def spmm_pass(tbl_ap, F, outs, uname, img128=None):
                """tbl_ap: [SHARD, F] gather table (DRAM). Produces the
                UNSCALED aggregate: partial -> per-phase ReduceScatter into
                outs = (phase0_out_ap, phase1_out_ap). dinv scalings happen
                in post-RS sweeps."""
                slot0 = [0]
                c1c = [0]
                c2c = [0]
                for ph in (0, 1):
                    rowbase = 0
                    pbuf = (partR if F == P else partAB)[ph]
                    for r in range(NC):
                        for bi, batch in enumerate(plan[ph][r]):
                            nt = sum(len(g[1]) for g in batch)
                            nch = nt + len(batch)
                            nidx = nch * P
                            graw = gpool.tile([P, 33 * 2 * P], bf16,
                                              name=f"g{uname}", tag="g")
                            g = (graw[:, :nch * F]
                                 .rearrange("p (c e) -> p c e", e=F))
                            seg = slot0[0] // 16
                            nc.gpsimd.dma_gather(
                                out_ap=g[:, :, :],
                                in_ap=tbl_ap,
                                idxs_ap=idx_sb[:, seg:seg + nidx // 16],
                                num_idxs=nidx,
                                num_idxs_reg=nreg2 if nch == 33 else nreg1,
                                elem_size=F,
                                single_packet=False,
                            )
                            nc1b = nt
                            ngrp = len(batch)
                            sel1 = spool.tile([P, 26, P], bf16,
                                              name=f"s1{uname}", tag="s1")
                            nc.vector.tensor_tensor(
                                out=sel1[:, :nc1b, :],
                                in0=iota1_sb[:, None, :]
                                    .to_broadcast([P, nc1b, P]),
                                in1=ld1_sb[:, c1c[0]:c1c[0] + nc1b, None]
                                    .to_broadcast([P, nc1b, P]),
                                op=AOP.is_equal)
                            sel2a = spool.tile([P, 7, 2 * P], bf16,
                                               name=f"s2a{uname}", tag="s2a")
                            nc.vector.tensor_tensor(
                                out=sel2a[:, :ngrp, :],
                                in0=iota2_sb[:, None, :]
                                    .to_broadcast([P, ngrp, 2 * P]),
                                in1=ld2a_sb[:, c2c[0]:c2c[0] + ngrp, None]
                                    .to_broadcast([P, ngrp, 2 * P]),
                                op=AOP.is_equal)
                            nquad = sum(1 for k, _ in batch if k == "q")
                            sel2b = spool.tile([P, 7, 2 * P], bf16,
                                               name=f"s2b{uname}", tag="s2b")
                            if nquad:
                                nc.vector.tensor_tensor(
                                    out=sel2b[:, :nquad, :],
                                    in0=iota2_sb[:, None, :]
                                        .to_broadcast([P, nquad, 2 * P]),
                                    in1=ld2b_sb[:, c2c[0]:c2c[0] + nquad, None]
                                        .to_broadcast([P, nquad, 2 * P]),
                                    op=AOP.is_equal)
                            stg = stgpool.tile([P, 26 * 2 * P], bf16,
                                               name=f"st{uname}", tag="st")
                            gpos = 0
                            s1pos = 0
                            tpos = 0
                            ndr = 0
                            for gi, (kind, lts) in enumerate(batch):
                                ntl = len(lts)
                                c2pos = gpos + ntl
                                tpg = (2 * P) // F   # tiles per psum group
                                psb = None
                                for j, lt in enumerate(lts):
                                    if j % tpg == 0:
                                        psb = ps_sp.tile([P, 2 * P], f32,
                                                         name=f"ps{uname}",
                                                         tag="ps",
                                                         space="PSUM")
                                    k = j % tpg
                                    ps = psb[:, k * F:(k + 1) * F]
                                    nc.tensor.matmul(
                                        ps, lhsT=sel1[:, s1pos + j, :],
                                        rhs=g[:, gpos + j, :],
                                        start=True, stop=False)
                                    selc = sel2a if j < 2 else sel2b
                                    half = (j & 1) * P
                                    nc.tensor.matmul(
                                        ps,
                                        lhsT=selc[:, gi, half:half + P],
                                        rhs=g[:, c2pos, :],
                                        start=False, stop=True)
                                    if j % tpg == tpg - 1 or j == ntl - 1:
                                        # drain k+1 tiles, scaling by the
                                        # global dinv image (dinv^1 for
                                        # single passes; interleaved
                                        # dinv^1|dinv^2 for A|B halves of
                                        # paired passes)
                                        ntile = k + 1
                                        nls = ntile * F
                                        gt0 = r * RT + lts[j - k]
                                        dst3 = (stg[:, tpos * F:
                                                    tpos * F + nls]
                                                .rearrange("p (i f) -> p i f",
                                                           f=P))
                                        nhf = nls // P
                                        if F == P:
                                            im = (img128 if img128 is not None
                                                  else dg1_sb)
                                            img = im[:, gt0:gt0 + nhf, None]
                                        else:
                                            img = dgAB_sb[:, 2 * gt0:
                                                          2 * gt0 + nhf,
                                                          None]
                                        src3 = (psb[:, :nls]
                                                .rearrange("p (i f) -> p i f",
                                                           f=P))
                                        nc.vector.tensor_tensor(
                                            out=dst3, in0=src3,
                                            in1=img.to_broadcast(
                                                [P, nhf, P]),
                                            op=AOP.mult)
                                        ndr += 1
                                        tpos += ntile
                                gpos += ntl + 1
                                s1pos += ntl
                            # stage -> partial rows
                            rview = pbuf[rowbase:rowbase + nt * P, :]
                            nc.sync.dma_start(
                                rview.rearrange("(i p) f -> p i f", p=P),
                                stg[:, :nt * F]
                                .rearrange("p (i f) -> p i f", f=F))
                            rowbase += nt * P
                            slot0[0] += nidx
                            c1c[0] += nc1b
                            c2c[0] += ngrp
                    # RS for this phase
                    nc.gpsimd.collective_compute(
                        "ReduceScatter", AOP.add,
                        replica_groups=[list(range(NC))],
                        ins=[pbuf[:]], outs=[outs[ph]])

            def out_slices(buf):
                return (buf[0:6144, :], buf[6144:SHARD, :])

            # ------------------------------------------------ helpers
            def sweep(src, W, items):
                """Node-major dinv scaling: load src [SHARD, W] in 8-tile
                batches; for each (col0, dimg, dst_tensor, dcol0, f32out):
                dst rows = dimg-col(tile) * src[:, col0:col0+P]."""
                for t0 in range(0, RT, 8):
                    ntl = min(8, RT - t0)
                    ld = wpool.tile([P, 8, 2 * P], bf16, name="ssl",
                                    tag="ssl")
                    sv = src[t0 * P:(t0 + ntl) * P, :]
                    nc.sync.dma_start(
                        ld[:, :ntl, :W],
                        sv.rearrange("(i p) f -> p i f", p=P))
                    for col0, dimg, dst, dcol0, f32out in items:
                        dt_ = f32 if f32out else bf16
                        so = wpool.tile([P, 8, P], dt_, name="sso",
                                        tag=f"sso{1 if f32out else 0}")
                        if dimg is None:
                            nc.scalar.activation(
                                out=so[:, :ntl, :],
                                in_=ld[:, :ntl, col0:col0 + P],
                                func=AF.Copy)
                        else:
                            nc.vector.tensor_tensor(
                                out=so[:, :ntl, :],
                                in0=ld[:, :ntl, col0:col0 + P],
                                in1=dimg[:, t0:t0 + ntl, None]
                                    .to_broadcast([P, ntl, P]),
                                op=AOP.mult)
                        dv = dst[t0 * P:(t0 + ntl) * P, dcol0:dcol0 + P]
                        nc.sync.dma_start(
                            dv.rearrange("(i p) f -> p i f", p=P),
                            so[:, :ntl, :])

            st = {}

            def stat_sweep(buf, col0, keyS, keyQ):
                stS = st[keyS]; stQ = st[keyQ]
                for grp in range(NG):
                    n0 = grp * GSZ
                    w = min(GSZ, SHARD - n0)
                    tT = hpool.tile([P, GSZ], bf16, name="swT", tag="swT")
                    nc.sync.dma_start_transpose(
                        tT[:, :w], buf[n0:n0 + w, col0:col0 + P])
                    nc.vector.reduce_sum(out=stS[:, grp:grp + 1],
                                         in_=tT[:, :w],
                                         axis=mybir.AxisListType.X)
                    sq = wpool.tile([P, GSZ], f32, name="swq", tag="swq")
                    nc.vector.tensor_tensor(out=sq[:, :w], in0=tT[:, :w],
                                            in1=tT[:, :w], op=AOP.mult)
                    nc.vector.reduce_sum(out=stQ[:, grp:grp + 1],
                                         in_=sq[:, :w],
                                         axis=mybir.AxisListType.X)

            def bn_reduce_and_AB(l, keys):
                ar = wpool.tile([P, 8], f32, name=f"ar{l}", tag="ar")
                for b in range(3):
                    nc.vector.reduce_sum(out=ar[:, b:b + 1],
                                         in_=st[keys[2 * b]][:],
                                         axis=mybir.AxisListType.X)
                    nc.vector.reduce_sum(out=ar[:, 3 + b:4 + b],
                                         in_=st[keys[2 * b + 1]][:],
                                         axis=mybir.AxisListType.X)
                nc.sync.dma_start(arin[l][:], ar[:])
                nc.gpsimd.collective_compute(
                    "AllReduce", AOP.add, replica_groups=[list(range(NC))],
                    ins=[arin[l][:]], outs=[arout[l][:]])
                gg = wpool.tile([P, 8], f32, name=f"arg{l}", tag="ar")
                nc.sync.dma_start(gg[:], arout[l][:])
                A = stpool.tile([P, 3], f32, name=f"A{l}")
                B = stpool.tile([P, 3], f32, name=f"B{l}")
                mu = wpool.tile([P, 3], f32, name=f"mu{l}", tag="mu")
                va = wpool.tile([P, 3], f32, name=f"va{l}", tag="mu")
                nc.vector.tensor_scalar(out=mu[:], in0=gg[:, 0:3],
                                        scalar1=1.0 / N, scalar2=None,
                                        op0=AOP.mult)
                nc.vector.tensor_scalar(out=va[:], in0=gg[:, 3:6],
                                        scalar1=1.0 / N, scalar2=None,
                                        op0=AOP.mult)
                musq = wpool.tile([P, 3], f32, name=f"ms{l}", tag="mu")
                nc.vector.tensor_tensor(out=musq[:], in0=mu[:], in1=mu[:],
                                        op=AOP.mult)
                nc.vector.tensor_tensor(out=va[:], in0=va[:], in1=musq[:],
                                        op=AOP.subtract)
                ve = wpool.tile([P, 3], f32, name=f"ve{l}", tag="mu")
                nc.vector.tensor_scalar(out=ve[:], in0=va[:],
                                        scalar1=float(EPS), scalar2=None,
                                        op0=AOP.add)
                sq_ = wpool.tile([P, 3], f32, name=f"sv{l}", tag="mu")
                nc.scalar.activation(out=sq_[:], in_=ve[:], func=AF.Sqrt)
                rs = wpool.tile([P, 3], f32, name=f"rs{l}", tag="mu")
                nc.vector.reciprocal(out=rs[:], in_=sq_[:])
                muA = wpool.tile([P, 3], f32, name=f"ma{l}", tag="mu")
                for b in range(3):
                    nc.vector.tensor_tensor(out=A[:, b:b + 1],
                                            in0=rs[:, b:b + 1],
                                            in1=bng(l, b), op=AOP.mult)
                    nc.vector.tensor_tensor(out=muA[:, b:b + 1],
                                            in0=mu[:, b:b + 1],
                                            in1=A[:, b:b + 1], op=AOP.mult)
                    nc.vector.tensor_tensor(out=B[:, b:b + 1],
                                            in0=bnb(l, b),
                                            in1=muA[:, b:b + 1],
                                            op=AOP.subtract)
                return A, B

            # ================================================== pipeline
            if phase_sel == "t1":
                spmm_pass(xtbl_t[:], P, out_slices(u1buf), "u1")
                sweep(u1buf, P, [(0, None, out_t, 0, True)])

            if phase_sel == "full":
                for k in ("S0", "Q0", "S1", "Q1", "S2", "Q2"):
                    st[(1, k)] = stpool.tile([P, NG], f32, name=f"st1{k}")
                    st[(2, k)] = stpool.tile([P, NG], f32, name=f"st2{k}")

                # -------- L1 spmm chain
                # u1 drains scale by dinv^2 -> RS output IS tbl2
                spmm_pass(xtbl_t[:], P, out_slices(tbl2), "u1",
                          img128=dg2_sb)
                # y1 = dinv*R1 = tbl2 / dinv (off critical path)
                sweep(tbl2, P, [(0, dlocI_sb, y1buf, 0, False)])
                # u2 drains scale by dinv^1 -> RS output IS y2
                spmm_pass(tbl2[:], P, out_slices(y2buf), "u2")

                # -------- L1 transform (hop-outer so hop-0 work runs
                # during the u-passes; hop-2 gates only its own tail)
                def l1_hop(hop):
                    for grp in range(NG):
                        n0 = grp * GSZ
                        w = min(GSZ, SHARD - n0)
                        yT = hpool.tile([P, GSZ], bf16, name="yT",
                                        tag=f"yT{hop}")
                        if hop == 0:
                            nc.sync.dma_start(yT[:, :w], xt_t[:, n0:n0 + w])
                        else:
                            ybuf = y1buf if hop == 1 else y2buf
                            nc.sync.dma_start_transpose(yT[:, :w],
                                                        ybuf[n0:n0 + w, :])
                        ps1 = ps_tf.tile([P, GSZ], f32, name="tf1", tag="tf",
                                         space="PSUM")
                        nc.tensor.matmul(ps1[:, :w],
                                         lhsT=w0_sb[:, hop * P:(hop + 1) * P],
                                         rhs=yT[:, :w], start=True,
                                         stop=False)
                        which = 2 if hop == 0 else hop - 1
                        nc.tensor.matmul(ps1[:, :w], lhsT=brow(0, hop),
                                         rhs=load_row(which, n0, w, "rs"),
                                         start=False, stop=True)
                        nc.vector.reduce_sum(
                            out=st[(1, f"S{hop}")][:, grp:grp + 1],
                            in_=ps1[:, :w], axis=mybir.AxisListType.X)
                        cp2 = hpool.tile([P, GSZ], bf16, name="cpt2",
                                         tag="cpt")
                        nc.scalar.activation(out=cp2[:, :w], in_=ps1[:, :w],
                                             func=AF.Copy)
                        sq2 = wpool.tile([P, GSZ], f32, name="sqt2",
                                         tag="sqt")
                        nc.vector.tensor_tensor(out=sq2[:, :w],
                                                in0=cp2[:, :w],
                                                in1=cp2[:, :w], op=AOP.mult)
                        nc.vector.reduce_sum(
                            out=st[(1, f"Q{hop}")][:, grp:grp + 1],
                            in_=sq2[:, :w], axis=mybir.AxisListType.X)
                        nc.sync.dma_start(raw1[hop][:, n0:n0 + w],
                                          cp2[:, :w])
                l1_hop(0)
                l1_hop(1)
                l1_hop(2)
                A1, B1 = bn_reduce_and_AB(
                    0, [(1, k) for k in ("S0", "Q0", "S1", "Q1", "S2", "Q2")])

                # -------- transform L2 / L3
                def transform_layer(l, A, B, final):
                    # block sources: ("fm", buf [P,SHARD]) feature-major or
                    # ("nm", buf [SHARD,P]) node-major (transpose-load)
                    if l == 1:
                        srcs = [("fm", raw1[0], 0), ("fm", raw1[1], 0),
                                ("fm", raw1[2], 0)]
                    else:
                        srcs = [("fm", raw20, 0), ("nm", rab[0], 0),
                                ("nm", rb2[0], 0)]
                    for grp in range(NG):
                        n0 = grp * GSZ
                        w = min(GSZ, SHARD - n0)
                        nq = w // P
                        hbt = []
                        for b in range(3):
                            kind_b, src, c0 = srcs[b]
                            raw = hpool.tile([P, GSZ], bf16, name="raw",
                                             tag=f"raw{b}")
                            if kind_b == "fm":
                                nc.sync.dma_start(raw[:, :w],
                                                  src[:, n0:n0 + w])
                            else:
                                nc.sync.dma_start_transpose(
                                    raw[:, :w],
                                    src[n0:n0 + w, c0:c0 + P])
                            h = hpool.tile([P, GSZ], bf16, name="hh",
                                           tag=f"h{b}")
                            nc.scalar.activation(out=h[:, :w],
                                                 in_=raw[:, :w],
                                                 func=AF.Relu,
                                                 bias=B[:, b:b + 1],
                                                 scale=A[:, b:b + 1])
                            hbt.append(h)
                        for j in range(3):
                            ps = ps_tf.tile([P, GSZ], f32, name="tfj",
                                            tag="tf", space="PSUM")
                            for b in range(3):
                                nc.tensor.matmul(ps[:, :w],
                                                 lhsT=wblk(l, j, b),
                                                 rhs=hbt[b][:, :w],
                                                 start=(b == 0), stop=False)
                            nc.tensor.matmul(ps[:, :w], lhsT=brow(l, j),
                                             rhs=load_row(2, n0, w, "ro"),
                                             start=False, stop=True)
                            if j == 0 and not final:
                                nc.vector.reduce_sum(
                                    out=st[(2, "S0")][:, grp:grp + 1],
                                    in_=ps[:, :w], axis=mybir.AxisListType.X)
                                cp = hpool.tile([P, GSZ], bf16, name="cpj",
                                                tag="cpt")
                                nc.scalar.activation(out=cp[:, :w],
                                                     in_=ps[:, :w],
                                                     func=AF.Copy)
                                sq = wpool.tile([P, GSZ], f32, name="sqj",
                                                tag="sqt")
                                nc.vector.tensor_tensor(
                                    out=sq[:, :w], in0=cp[:, :w],
                                    in1=cp[:, :w], op=AOP.mult)
                                nc.vector.reduce_sum(
                                    out=st[(2, "Q0")][:, grp:grp + 1],
                                    in_=sq[:, :w], axis=mybir.AxisListType.X)
                                nc.sync.dma_start(raw20[:, n0:n0 + w],
                                                  cp[:, :w])
                            elif j == 0 and final:
                                for q in range(nq):
                                    cpf = wpool.tile([P, P], f32, name="cpf",
                                                     tag="cpf")
                                    nc.vector.tensor_copy(
                                        out=cpf[:],
                                        in_=ps[:, q * P:(q + 1) * P])
                                    pst = ps_tp.tile([P, P], f32, name="ptf",
                                                     tag="tpf", space="PSUM",
                                                     bufs=1)
                                    nc.tensor.transpose(pst[:], cpf[:],
                                                        identf_sb)
                                    of = wpool.tile([P, P], f32, name="of",
                                                    tag="cpf")
                                    nc.scalar.activation(out=of[:],
                                                         in_=pst[:],
                                                         func=AF.Copy)
                                    nc.sync.dma_start(
                                        out_t[n0 + q * P:n0 + (q + 1) * P,
                                              0:P], of[:])
                            else:
                                # t_j -> transpose -> dloc1-scale -> table
                                dst = tblpair[l - 1]
                                stgt = stgpool.tile([P, 4, P], bf16,
                                                    name="tstg",
                                                    tag=f"tstg{j}")
                                for q in range(nq):
                                    cpb = wpool.tile([P, P], bf16,
                                                     name="cpb", tag="cpb")
                                    nc.vector.tensor_copy(
                                        out=cpb[:],
                                        in_=ps[:, q * P:(q + 1) * P])
                                    pst = ps_tp.tile([P, P], bf16,
                                                     name="ptb", tag="tp",
                                                     space="PSUM")
                                    nc.tensor.transpose(pst[:], cpb[:],
                                                        ident_sb)
                                    tt = n0 // P + q
                                    nc.vector.tensor_scalar(
                                        out=stgt[:, q, :], in0=pst[:],
                                        scalar1=dloc1_sb[:, tt:tt + 1],
                                        scalar2=None, op0=AOP.mult)
                                dv = dst[n0:n0 + w,
                                         (j - 1) * P:j * P]
                                nc.sync.dma_start(
                                    dv.rearrange("(i p) f -> p i f", p=P),
                                    stgt[:, :nq, :])

                # L2
                transform_layer(1, A1, B1, final=False)
                spmm_pass(tblpair[0][:], 2 * P, out_slices(rab[0]), "v1")
                sweep(rab[0], 2 * P, [(P, None, tblu[0], 0, False)])
                spmm_pass(tblu[0][:], P, out_slices(rb2[0]), "v2")
                stat_sweep(rab[0], 0, (2, "S1"), (2, "Q1"))
                stat_sweep(rb2[0], 0, (2, "S2"), (2, "Q2"))
                A2, B2 = bn_reduce_and_AB(
                    1, [(2, k) for k in ("S0", "Q0", "S1", "Q1", "S2", "Q2")])

                # L3
                transform_layer(2, A2, B2, final=True)
                spmm_pass(tblpair[1][:], 2 * P, out_slices(rab[1]), "w1")
                sweep(rab[1], 2 * P, [(P, None, tblu[1], 0, False)])
                spmm_pass(tblu[1][:], P, out_slices(rb2[1]), "w2")
                sweep(rab[1], 2 * P, [(0, None, out_t, P, True)])
                sweep(rb2[1], P, [(0, None, out_t, 2 * P, True)])

    if not for_sim:
        _split_excess_waits(nc)
        mybir.codegen_inst_isa_subclasses(nc)
    return nc


_CACHE = {}


def kernel(x, edge_index, W0, b0, W1, b1, W2, b2, bn_gamma, bn_beta):
    x = np.asarray(x, np.float32)
    edge_index = np.asarray(edge_index)
    cores = _host_prep(x, edge_index)

    W0 = np.asarray(W0, np.float32)
    W1 = np.asarray(W1, np.float32)
    W2 = np.asarray(W2, np.float32)
    b0 = np.asarray(b0, np.float32)
    b1 = np.asarray(b1, np.float32)
    b2 = np.asarray(b2, np.float32)
    bn_g = np.asarray(bn_gamma, np.float32)
    bn_b = np.asarray(bn_beta, np.float32)

    in_maps = []
    for c in range(NC):
        cb, cr, cf = _pack_consts(cores[c], W0, b0, W1, b1, W2, b2,
                                  bn_g, bn_b)
        in_maps.append(dict(
            xtbl=cores[c]["xtbl"], xt=cores[c]["xt"], idx=cores[c]["idx16"],
            cb=cb, cr=cr, cf=cf))

    phase = os.environ.get("KPHASE", "full")
    if phase not in _CACHE:
        _CACHE[phase] = _build(phase)
    nc = _CACHE[phase]
    res = run_bass_kernel_spmd(nc, in_maps, core_ids=list(range(NC)),
                               trace=bool(os.environ.get("KERNEL_TRACE")))
    global last_result
    last_result = res
    out = np.concatenate([r["out"] for r in res.results], axis=0)
    return out[:N].astype(np.float32)


last_result = None


# revision 3
# speedup vs baseline: 1.0576x; 1.0576x over previous
"""MixHop GNN v2: source-stationary SpMM + ReduceScatter on 8 trn2 cores.

vs v1 (gather-from-replicated-table + AllGather):
 - Each core owns src shard [12544 rows]; every SpMM gathers ONLY from the
   local-shard table (int16 idx, no quartering) and produces a partial for
   ALL 784 dst tiles; a ReduceScatter(add) returns the local shard of the
   aggregate. RS is ~2-3x cheaper than AG in the cost model and is split
   into 2 phase-halves that overlap with compute.
 - Edge slots: per rank-of-98-tiles: 24 quads (4 tiles) + 1 pair; each tile
   has one 128-slot c1 chunk; each group has ONE shared 128-slot overflow
   chunk (c2). 125952 slots vs 150528 (v1) for the same edges.
 - dinv[dst] scaling folded into the PSUM drains (pre-RS; commutes with the
   sum), so RS outputs are directly h-ready or table-ready (dinv^2).
 - L2/L3 first hops share one pass: table [12544, 256] = [dinv*t1|dinv*t2],
   512B gather elements (no <512B DMA penalty), one sel serves both.
 - Transposes via HWDGE dma_start_transpose instead of PE+copy.
"""
import os
import numpy as np
import ml_dtypes

import concourse.bass as bass
import concourse.mybir as mybir
import concourse.tile as tile
from concourse import library_config
from concourse.bass_utils import run_bass_kernel_spmd

bf16 = mybir.dt.bfloat16
f32 = mybir.dt.float32
i16 = mybir.dt.int16
BF = ml_dtypes.bfloat16

N = 100000
NC = 8
P = 128
D = 128
SHARD = 12544
NPAD = SHARD * NC
RT = 98                  # tiles per rank
NQ = 24                  # quads per rank (plus 1 trailing pair)
PH1T = 48                # tiles in phase 1 (quads 0..11)
PH2T = 50                # tiles in phase 2 (quads 12..23 + pair)
GSZ = 512
NG = (SHARD + GSZ - 1) // GSZ   # 25 transform groups
EPS = 1e-5

# groups per rank: 24 quads + 1 pair; chunks: quads 5, pair 3
CPR = NQ * 5 + 3         # 123 chunks per rank
NSLOT = NC * CPR * P     # 125952
NC1 = 784                # c1 chunks total
NC2 = NC * (NQ + 1)      # 200 overflow chunks

AOP = mybir.AluOpType
AF = mybir.ActivationFunctionType

_SKIP_WAITSPLIT = (mybir.InstEventSemaphore,)


def _split_excess_waits(nc, keep=1):
    n = 0
    uid = [0]
    for fn in nc.m.functions:
        for blk in fn.blocks:
            insts = list(blk.instructions)
            out = []
            for inst in insts:
                si = inst.sync_info
                if (si is not None and si.on_wait and len(si.on_wait) > keep
                        and not isinstance(inst, _SKIP_WAITSPLIT)):
                    waits = list(si.on_wait)
                    extra, rest = waits[:-keep], waits[-keep:]
                    for w in extra:
                        uid[0] += 1
                        out.append(mybir.InstEventSemaphore(
                            name=f"evws_{uid[0]}",
                            engine=inst.engine,
                            ins=[], outs=[],
                            sync_info=mybir.SyncInfo(on_wait=[w], on_update=[]),
                        ))
                        n += 1
                    inst.sync_info = mybir.SyncInfo(
                        on_wait=rest, on_update=list(si.on_update or []))
                out.append(inst)
            if len(out) != len(insts):
                blk.instructions = out
    return n


# ------------------------------------------------------------------ batches
def batch_plan():
    """Static per-(phase, rank) batch structure.

    Returns list over phases of list over ranks of batches; each batch is a
    list of groups; each group is (kind, tiles) with kind in {"q","p"} and
    tiles = local tile indices (lt) in ascending order.
    """
    phases = []
    for ph in (0, 1):
        ranks = []
        for r in range(NC):
            if ph == 0:
                quads = [("q", [4 * q + i for i in range(4)])
                         for q in range(12)]
                batches = [quads[:6], quads[6:]]
            else:
                quads = [("q", [4 * q + i for i in range(4)])
                         for q in range(12, 24)]
                pair = ("p", [96, 97])
                batches = [quads[:6], quads[6:] + [pair]]
            ranks.append(batches)
        phases.append(ranks)
    return phases


# ---------------------------------------------------------------- host prep
def _host_prep(x, edge_index):
    row = edge_index[0].astype(np.int64)
    col = edge_index[1].astype(np.int64)
    deg = np.bincount(col, minlength=N).astype(np.float64)
    dinv = np.where(deg > 0, 1.0 / np.sqrt(np.maximum(deg, 1.0)), 0.0)
    s1 = dinv * np.bincount(col, weights=dinv[row], minlength=N)
    s2 = dinv * np.bincount(col, weights=dinv[row] * s1[row], minlength=N)

    dinv_pad = np.zeros(NPAD)
    dinv_pad[:N] = dinv
    x_pad = np.zeros((NPAD, D), np.float32)
    x_pad[:N] = x
    s1_pad = np.zeros(NPAD, np.float32)
    s1_pad[:N] = s1
    s2_pad = np.zeros(NPAD, np.float32)
    s2_pad[:N] = s2
    ones_pad = np.zeros(NPAD, np.float32)
    ones_pad[:N] = 1.0

    plan = batch_plan()
    cores = []
    for c in range(NC):
        lo = c * SHARD
        m = (row >= lo) & (row < lo + SHARD)
        r_l = (row[m] - lo).astype(np.int64)
        cc = col[m]
        t = cc >> 7
        order = np.argsort(t, kind="stable")
        r_l, cc, t = r_l[order], cc[order], t[order]
        cnt = np.bincount(t, minlength=784)
        starts = np.zeros(785, np.int64)
        starts[1:] = np.cumsum(cnt)

        idx = np.zeros(NSLOT, np.int16)
        ld1 = np.full((P, NC1), -1.0, np.float32)
        ld2a = np.full((P, NC2), -1.0, np.float32)
        ld2b = np.full((P, NC2), -1.0, np.float32)
        slot = 0
        c1i = 0
        c2i = 0
        for ph in (0, 1):
            for r in range(NC):
                for batch in plan[ph][r]:
                    for kind, lts in batch:
                        ov_idx = []
                        ov_rel = []
                        for j, lt in enumerate(lts):
                            gt = r * RT + lt
                            s, e = starts[gt], starts[gt + 1]
                            n1 = min(e - s, P)
                            idx[slot:slot + n1] = r_l[s:s + n1]
                            ld1[:n1, c1i] = (cc[s:s + n1] & 127)
                            slot += P
                            c1i += 1
                            if e - s > n1:
                                ov_idx.append(r_l[s + n1:e])
                                ov_rel.append((cc[s + n1:e] & 127) + P * j)
                        ov_idx = (np.concatenate(ov_idx) if ov_idx
                                  else np.zeros(0, np.int64))
                        ov_rel = (np.concatenate(ov_rel) if ov_rel
                                  else np.zeros(0, np.int64))
                        no = len(ov_idx)
                        assert no <= P, f"overflow {no} > 128"
                        idx[slot:slot + no] = ov_idx
                        ra = ov_rel.astype(np.float32)
                        ld2a[:no, c2i] = np.where(ov_rel < 256, ra, -1.0)
                        ld2b[:no, c2i] = np.where(ov_rel >= 256, ra - 256, -1.0)
                        slot += P
                        c2i += 1
        assert slot == NSLOT and c1i == NC1 and c2i == NC2

        # idx16: per batch segment, 16-wrap + 8x replicate
        idx16 = np.zeros((P, NSLOT // 16), np.int16)
        pos = 0
        for ph in (0, 1):
            for r in range(NC):
                for batch in plan[ph][r]:
                    nch = sum(len(g[1]) + 1 for g in batch)
                    nidx = nch * P
                    seg = idx[pos:pos + nidx]
                    blk = seg.reshape(-1, 16).T
                    idx16[:, pos // 16:(pos + nidx) // 16] = np.tile(blk, (8, 1))
                    pos += nidx

        dl = dinv_pad[lo:lo + SHARD]
        xtbl = (dl[:, None] * x_pad[lo:lo + SHARD]).astype(BF)
        xt = x_pad[lo:lo + SHARD].T.astype(BF)
        dloc1 = dl.reshape(RT, P).T.astype(np.float32)
        dloc2 = (dl ** 2).reshape(RT, P).T.astype(np.float32)
        dlI = np.where(dl > 0, 1.0 / np.maximum(dl, 1e-30), 0.0)
        dlocI = dlI.reshape(RT, P).T.astype(np.float32)
        dg1 = dinv_pad.reshape(784, P).T.astype(np.float32)
        dg2 = (dinv_pad ** 2).reshape(784, P).T.astype(np.float32)
        dgAB = np.empty((P, 784 * 2), np.float32)
        dgAB[:, 0::2] = dg1
        dgAB[:, 1::2] = dg2
        srow = np.concatenate([s1_pad[lo:lo + SHARD], s2_pad[lo:lo + SHARD]])
        onesr = ones_pad[lo:lo + SHARD]
        cores.append(dict(idx16=idx16, ld1=ld1, ld2a=ld2a, ld2b=ld2b,
                          xtbl=xtbl, xt=xt, dloc1=dloc1, dloc2=dloc2,
                          dlocI=dlocI, dg1=dg1, dgAB=dgAB,
                          srow=srow[None, :].astype(BF),
                          onesr=onesr[None, :].astype(BF)))
    return cores


def _pack_consts(core, W0, b0, W1, b1, W2, b2, bn_g, bn_b):
    iota1 = np.tile(np.arange(P, dtype=np.float32), (P, 1)).astype(BF)
    iota2 = np.tile(np.arange(2 * P, dtype=np.float32), (P, 1)).astype(BF)
    ident = np.eye(P, dtype=np.float32)
    w0c = np.concatenate([W0[j] for j in range(3)], axis=1)
    blocks = []
    for W in (W1, W2):
        for j in range(3):
            for b in range(3):
                blocks.append(W[j][b * P:(b + 1) * P, :])
    w12c = np.concatenate(blocks, axis=1)
    cb = np.concatenate([iota1, iota2, core["ld1"].astype(BF),
                         core["ld2a"].astype(BF), core["ld2b"].astype(BF),
                         ident.astype(BF), w0c.astype(BF), w12c.astype(BF)],
                        axis=1)
    br = np.concatenate([b0.reshape(1, -1), b1.reshape(1, -1),
                         b2.reshape(1, -1)], axis=1)
    cr = np.concatenate([br.astype(np.float32),
                         core["srow"].astype(np.float32),
                         core["onesr"].astype(np.float32)], axis=1).astype(BF)
    bng = np.stack([bn_g[l].reshape(3, P).T for l in range(2)], axis=0)
    bnb = np.stack([bn_b[l].reshape(3, P).T for l in range(2)], axis=0)
    cf = np.concatenate([core["dloc1"], core["dloc2"], core["dlocI"],
                         bng[0], bng[1], bnb[0], bnb[1], ident,
                         core["dg1"], core["dgAB"]], axis=1)
    return cb.astype(BF), cr, cf.astype(np.float32)


NCB = P + 2 * P + NC1 + 2 * NC2 + P + 3 * P + 18 * P
NCR = 9 * P + 2 * SHARD + SHARD
NCF = 3 * RT + 12 + P + 784 * 3


# ---------------------------------------------------------------- device
def _build(phase_sel="full", for_sim=False):
    plan = batch_plan()
    nc = bass.Bass(num_devices=NC)
    xtbl_t = nc.dram_tensor("xtbl", [SHARD, P], bf16, kind="ExternalInput")
    xt_t = nc.dram_tensor("xt", [P, SHARD], bf16, kind="ExternalInput")
    idx_t = nc.dram_tensor("idx", [P, NSLOT // 16], i16, kind="ExternalInput")
    cb_t = nc.dram_tensor("cb", [P, NCB], bf16, kind="ExternalInput")
    cr_t = nc.dram_tensor("cr", [1, NCR], bf16, kind="ExternalInput")
    cf_t = nc.dram_tensor("cf", [P, NCF], f32, kind="ExternalInput")
    out_t = nc.dram_tensor("out", [SHARD, 3 * P], f32, kind="ExternalOutput")

    with tile.TileContext(nc) as tc:
        with (
            tc.tile_pool(name="const", bufs=1) as cpool,
            tc.tile_pool(name="gath", bufs=2) as gpool,
            tc.tile_pool(name="sel", bufs=2) as spool,
            tc.tile_pool(name="stg", bufs=2) as stgpool,
            tc.tile_pool(name="work", bufs=3) as wpool,
            tc.tile_pool(name="hb", bufs=2) as hpool,
            tc.tile_pool(name="stat", bufs=1) as stpool,
            tc.tile_pool(name="psp", bufs=3, space="PSUM") as ps_sp,
            tc.tile_pool(name="ptf", bufs=2, space="PSUM") as ps_tf,
            tc.tile_pool(name="ptp", bufs=2, space="PSUM") as ps_tp,
            tc.tile_pool(name="dram", bufs=1, space="DRAM") as dpool,
        ):
            nc.gpsimd.load_library(library_config.mlp)

            idx_sb = cpool.tile([P, NSLOT // 16], i16)
            nc.sync.dma_start(idx_sb[:], idx_t[:])
            cb_sb = cpool.tile([P, NCB], bf16)
            nc.sync.dma_start(cb_sb[:], cb_t[:])
            bias_sb = cpool.tile([1, 9 * P], bf16)
            nc.sync.dma_start(bias_sb[:], cr_t[:1, :9 * P])
            cf_sb = cpool.tile([P, NCF], f32)
            nc.sync.dma_start(cf_sb[:], cf_t[:])

            o = 0
            iota1_sb = cb_sb[:, o:o + P]; o += P
            iota2_sb = cb_sb[:, o:o + 2 * P]; o += 2 * P
            ld1_sb = cb_sb[:, o:o + NC1]; o += NC1
            ld2a_sb = cb_sb[:, o:o + NC2]; o += NC2
            ld2b_sb = cb_sb[:, o:o + NC2]; o += NC2
            ident_sb = cb_sb[:, o:o + P]; o += P
            w0_sb = cb_sb[:, o:o + 3 * P]; o += 3 * P
            w12_sb = cb_sb[:, o:o + 18 * P]

            def wblk(l, j, b):
                i = ((l - 1) * 9 + j * 3 + b) * P
                return w12_sb[:, i:i + P]

            def brow(l, j):
                i = (l * 3 + j) * P
                return bias_sb[:1, i:i + P]

            def load_row(which, n0, w, tag):
                i = 9 * P + which * SHARD + n0
                rt_ = wpool.tile([1, GSZ], bf16, name="row", tag=tag)
                nc.sync.dma_start(rt_[:1, :w], cr_t[:1, i:i + w])
                return rt_[:1, :w]

            dloc1_sb = cf_sb[:, 0:RT]
            dloc2_sb = cf_sb[:, RT:2 * RT]
            dlocI_sb = cf_sb[:, 2 * RT:3 * RT]

            def bng(l, b):
                i = 3 * RT + l * 3 + b
                return cf_sb[:, i:i + 1]

            def bnb(l, b):
                i = 3 * RT + 6 + l * 3 + b
                return cf_sb[:, i:i + 1]

            identf_sb = cf_sb[:, 3 * RT + 12:3 * RT + 12 + P]
            _o2 = 3 * RT + 12 + P
            dg1_sb = cf_sb[:, _o2:_o2 + 784]
            dgAB_sb = cf_sb[:, _o2 + 784:_o2 + 784 * 3]
            dg2_sb = (dgAB_sb.rearrange("p (t two) -> p t two", two=2)
                      [:, :, 1])

            # DRAM buffers
            def dbuf(name, rows, colsw, dt=bf16):
                return dpool.tile([rows, colsw], dt, name=name)

            # partials: [*,128] for single passes, [*,256] for paired
            partR = [dbuf("pR1", NC * 6144, P), dbuf("pR2", NC * 6400, P)]
            partAB = [dbuf("pAB1", NC * 6144, 2 * P),
                      dbuf("pAB2", NC * 6400, 2 * P)]
            u1buf = dbuf("u1buf", SHARD, P)
            y1buf = dbuf("y1buf", SHARD, P)       # dinv*R1
            y2buf = dbuf("y2buf", SHARD, P)       # dinv*R2
            tbl2 = dbuf("tbl2", SHARD, P)         # dinv^2*R1
            tblpair = [dbuf(f"tblpair{l}", SHARD, 2 * P) for l in (0, 1)]
            rab = [dbuf(f"rab{l}", SHARD, 2 * P) for l in (0, 1)]
            tblu = [dbuf(f"tblu{l}", SHARD, P) for l in (0, 1)]
            rb2 = [dbuf(f"rb2_{l}", SHARD, P) for l in (0, 1)]
            raw1 = [dbuf(f"raw1_{b}", P, SHARD) for b in range(3)]
            raw20 = dbuf("raw2_0", P, SHARD)
            arin = [dpool.tile([P, 8], f32, name=f"arin{l}") for l in range(2)]
            arout = [dpool.tile([P, 8], f32, name=f"arout{l}",
                                addr_space="Shared") for l in range(2)]

            nreg1 = nc.gpsimd.to_reg(30 * P)
            nreg2 = nc.gpsimd.to_reg(33 * P)

            # ------------------------------------------------ spmm pass
            def spmm_pass(tbl_ap, F, outs, uname, img128=None):
                """tbl_ap: [SHARD, F] gather table (DRAM). Produces the
                UNSCALED aggregate: partial -> per-phase ReduceScatter into
                outs = (phase0_out_ap, phase1_out_ap). dinv scalings happen
                in post-RS sweeps."""
                slot0 = [0]
                c1c = [0]
                c2c = [0]
                for ph in (0, 1):
                    rowbase = 0
                    pbuf = (partR if F == P else partAB)[ph]
                    for r in range(NC):
                        for bi, batch in enumerate(plan[ph][r]):
                            nt = sum(len(g[1]) for g in batch)
                            nch = nt + len(batch)
                            nidx = nch * P
                            graw = gpool.tile([P, 33 * 2 * P], bf16,
                                              name=f"g{uname}", tag="g")
                            g = (graw[:, :nch * F]
                                 .rearrange("p (c e) -> p c e", e=F))
                            seg = slot0[0] // 16
                            nc.gpsimd.dma_gather(
                                out_ap=g[:, :, :],
                                in_ap=tbl_ap,
                                idxs_ap=idx_sb[:, seg:seg + nidx // 16],
                                num_idxs=nidx,
                                num_idxs_reg=nreg2 if nch == 33 else nreg1,
                                elem_size=F,
                                single_packet=False,
                            )
                            nc1b = nt
                            ngrp = len(batch)
                            sel1 = spool.tile([P, 26, P], bf16,
                                              name=f"s1{uname}", tag="s1")
                            nc.vector.tensor_tensor(
                                out=sel1[:, :nc1b, :],
                                in0=iota1_sb[:, None, :]
                                    .to_broadcast([P, nc1b, P]),
                                in1=ld1_sb[:, c1c[0]:c1c[0] + nc1b, None]
                                    .to_broadcast([P, nc1b, P]),
                                op=AOP.is_equal)
                            sel2a = spool.tile([P, 7, 2 * P], bf16,
                                               name=f"s2a{uname}", tag="s2a")
                            nc.vector.tensor_tensor(
                                out=sel2a[:, :ngrp, :],
                                in0=iota2_sb[:, None, :]
                                    .to_broadcast([P, ngrp, 2 * P]),
                                in1=ld2a_sb[:, c2c[0]:c2c[0] + ngrp, None]
                                    .to_broadcast([P, ngrp, 2 * P]),
                                op=AOP.is_equal)
                            nquad = sum(1 for k, _ in batch if k == "q")
                            sel2b = spool.tile([P, 7, 2 * P], bf16,
                                               name=f"s2b{uname}", tag="s2b")
                            if nquad:
                                nc.vector.tensor_tensor(
                                    out=sel2b[:, :nquad, :],
                                    in0=iota2_sb[:, None, :]
                                        .to_broadcast([P, nquad, 2 * P]),
                                    in1=ld2b_sb[:, c2c[0]:c2c[0] + nquad, None]
                                        .to_broadcast([P, nquad, 2 * P]),
                                    op=AOP.is_equal)
                            stg = stgpool.tile([P, 26 * 2 * P], bf16,
                                               name=f"st{uname}", tag="st")
                            gpos = 0
                            s1pos = 0
                            tpos = 0
                            ndr = 0
                            for gi, (kind, lts) in enumerate(batch):
                                ntl = len(lts)
                                c2pos = gpos + ntl
                                tpg = (2 * P) // F   # tiles per psum group
                                psb = None
                                for j, lt in enumerate(lts):
                                    if j % tpg == 0:
                                        psb = ps_sp.tile([P, 2 * P], f32,
                                                         name=f"ps{uname}",
                                                         tag="ps",
                                                         space="PSUM")
                                    k = j % tpg
                                    ps = psb[:, k * F:(k + 1) * F]
                                    nc.tensor.matmul(
                                        ps, lhsT=sel1[:, s1pos + j, :],
                                        rhs=g[:, gpos + j, :],
                                        start=True, stop=False)
                                    selc = sel2a if j < 2 else sel2b
                                    half = (j & 1) * P
                                    nc.tensor.matmul(
                                        ps,
                                        lhsT=selc[:, gi, half:half + P],
                                        rhs=g[:, c2pos, :],
                                        start=False, stop=True)
                                    if j % tpg == tpg - 1 or j == ntl - 1:
                                        # drain k+1 tiles, scaling by the
                                        # global dinv image (dinv^1 for
                                        # single passes; interleaved
                                        # dinv^1|dinv^2 for A|B halves of
                                        # paired passes)
                                        ntile = k + 1
                                        nls = ntile * F
                                        gt0 = r * RT + lts[j - k]
                                        dst3 = (stg[:, tpos * F:
                                                    tpos * F + nls]
                                                .rearrange("p (i f) -> p i f",
                                                           f=P))
                                        nhf = nls // P
                                        if F == P:
                                            im = (img128 if img128 is not None
                                                  else dg1_sb)
                                            img = im[:, gt0:gt0 + nhf, None]
                                        else:
                                            img = dgAB_sb[:, 2 * gt0:
                                                          2 * gt0 + nhf,
                                                          None]
                                        src3 = (psb[:, :nls]
                                                .rearrange("p (i f) -> p i f",
                                                           f=P))
                                        if ndr % 3 == 0:
                                            nc.vector.tensor_tensor(
                                                out=dst3, in0=src3,
                                                in1=img.to_broadcast(
                                                    [P, nhf, P]),
                                                op=AOP.mult)
                                        else:
                                            for ti in range(ntile):
                                                gt = gt0 + ti
                                                if F == P:
                                                    im0 = (img128 if img128
                                                           is not None
                                                           else dg1_sb)
                                                    nc.scalar.activation(
                                                        out=dst3[:, ti, :],
                                                        in_=src3[:, ti, :],
                                                        func=AF.Copy,
                                                        scale=im0[:,
                                                                  gt:gt + 1])
                                                else:
                                                    nc.scalar.activation(
                                                        out=dst3[:, 2 * ti,
                                                                 :],
                                                        in_=src3[:, 2 * ti,
                                                                 :],
                                                        func=AF.Copy,
                                                        scale=dg1_sb[
                                                            :, gt:gt + 1])
                                                    nc.scalar.activation(
                                                        out=dst3[:,
                                                                 2 * ti + 1,
                                                                 :],
                                                        in_=src3[:,
                                                                 2 * ti + 1,
                                                                 :],
                                                        func=AF.Copy,
                                                        scale=dg2_sb[
                                                            :, gt:gt + 1])
                                        ndr += 1
                                        tpos += ntile
                                gpos += ntl + 1
                                s1pos += ntl
                            # stage -> partial rows
                            rview = pbuf[rowbase:rowbase + nt * P, :]
                            nc.sync.dma_start(
                                rview.rearrange("(i p) f -> p i f", p=P),
                                stg[:, :nt * F]
                                .rearrange("p (i f) -> p i f", f=F))
                            rowbase += nt * P
                            slot0[0] += nidx
                            c1c[0] += nc1b
                            c2c[0] += ngrp
                    # RS for this phase
                    nc.gpsimd.collective_compute(
                        "ReduceScatter", AOP.add,
                        replica_groups=[list(range(NC))],
                        ins=[pbuf[:]], outs=[outs[ph]])

            def out_slices(buf):
                return (buf[0:6144, :], buf[6144:SHARD, :])

            # ------------------------------------------------ helpers
            def sweep(src, W, items):
                """Node-major dinv scaling: load src [SHARD, W] in 8-tile
                batches; for each (col0, dimg, dst_tensor, dcol0, f32out):
                dst rows = dimg-col(tile) * src[:, col0:col0+P]."""
                for t0 in range(0, RT, 4):
                    ntl = min(4, RT - t0)
                    ld = wpool.tile([P, 4, 2 * P], bf16, name="ssl",
                                    tag="ssl")
                    sv = src[t0 * P:(t0 + ntl) * P, :]
                    nc.sync.dma_start(
                        ld[:, :ntl, :W],
                        sv.rearrange("(i p) f -> p i f", p=P))
                    for col0, dimg, dst, dcol0, f32out in items:
                        dt_ = f32 if f32out else bf16
                        so = wpool.tile([P, 4, P], dt_, name="sso",
                                        tag=f"sso{1 if f32out else 0}")
                        if dimg is None:
                            nc.scalar.activation(
                                out=so[:, :ntl, :],
                                in_=ld[:, :ntl, col0:col0 + P],
                                func=AF.Copy)
                        else:
                            nc.vector.tensor_tensor(
                                out=so[:, :ntl, :],
                                in0=ld[:, :ntl, col0:col0 + P],
                                in1=dimg[:, t0:t0 + ntl, None]
                                    .to_broadcast([P, ntl, P]),
                                op=AOP.mult)
                        dv = dst[t0 * P:(t0 + ntl) * P, dcol0:dcol0 + P]
                        nc.sync.dma_start(
                            dv.rearrange("(i p) f -> p i f", p=P),
                            so[:, :ntl, :])

            st = {}

            def stat_sweep(buf, col0, keyS, keyQ):
                stS = st[keyS]; stQ = st[keyQ]
                for grp in range(NG):
                    n0 = grp * GSZ
                    w = min(GSZ, SHARD - n0)
                    par = grp % 2
                    tT = hpool.tile([P, GSZ], bf16, name="swT",
                                    tag="cpt" if par == 0 else "yT")
                    nc.sync.dma_start_transpose(
                        tT[:, :w], buf[n0:n0 + w, col0:col0 + P])
                    nc.vector.reduce_sum(out=stS[:, grp:grp + 1],
                                         in_=tT[:, :w],
                                         axis=mybir.AxisListType.X)
                    sq = wpool.tile([P, GSZ], f32, name="swq",
                                    tag="swq" if par == 0 else "sqt")
                    nc.vector.tensor_tensor(out=sq[:, :w], in0=tT[:, :w],
                                            in1=tT[:, :w], op=AOP.mult)
                    nc.vector.reduce_sum(out=stQ[:, grp:grp + 1],
                                         in_=sq[:, :w],
                                         axis=mybir.AxisListType.X)

            def bn_reduce_and_AB(l, keys):
                ar = wpool.tile([P, 8], f32, name=f"ar{l}", tag="ar")
                for b in range(3):
                    nc.vector.reduce_sum(out=ar[:, b:b + 1],
                                         in_=st[keys[2 * b]][:],
                                         axis=mybir.AxisListType.X)
                    nc.vector.reduce_sum(out=ar[:, 3 + b:4 + b],
                                         in_=st[keys[2 * b + 1]][:],
                                         axis=mybir.AxisListType.X)
                nc.sync.dma_start(arin[l][:], ar[:])
                nc.gpsimd.collective_compute(
                    "AllReduce", AOP.add, replica_groups=[list(range(NC))],
                    ins=[arin[l][:]], outs=[arout[l][:]])
                gg = wpool.tile([P, 8], f32, name=f"arg{l}", tag="ar")
                nc.sync.dma_start(gg[:], arout[l][:])
                A = stpool.tile([P, 3], f32, name=f"A{l}")
                B = stpool.tile([P, 3], f32, name=f"B{l}")
                mu = wpool.tile([P, 3], f32, name=f"mu{l}", tag="mu")
                va = wpool.tile([P, 3], f32, name=f"va{l}", tag="mu")
                nc.vector.tensor_scalar(out=mu[:], in0=gg[:, 0:3],
                                        scalar1=1.0 / N, scalar2=None,
                                        op0=AOP.mult)
                nc.vector.tensor_scalar(out=va[:], in0=gg[:, 3:6],
                                        scalar1=1.0 / N, scalar2=None,
                                        op0=AOP.mult)
                musq = wpool.tile([P, 3], f32, name=f"ms{l}", tag="mu")
                nc.vector.tensor_tensor(out=musq[:], in0=mu[:], in1=mu[:],
                                        op=AOP.mult)
                nc.vector.tensor_tensor(out=va[:], in0=va[:], in1=musq[:],
                                        op=AOP.subtract)
                ve = wpool.tile([P, 3], f32, name=f"ve{l}", tag="mu")
                nc.vector.tensor_scalar(out=ve[:], in0=va[:],
                                        scalar1=float(EPS), scalar2=None,
                                        op0=AOP.add)
                sq_ = wpool.tile([P, 3], f32, name=f"sv{l}", tag="mu")
                nc.scalar.activation(out=sq_[:], in_=ve[:], func=AF.Sqrt)
                rs = wpool.tile([P, 3], f32, name=f"rs{l}", tag="mu")
                nc.vector.reciprocal(out=rs[:], in_=sq_[:])
                muA = wpool.tile([P, 3], f32, name=f"ma{l}", tag="mu")
                for b in range(3):
                    nc.vector.tensor_tensor(out=A[:, b:b + 1],
                                            in0=rs[:, b:b + 1],
                                            in1=bng(l, b), op=AOP.mult)
                    nc.vector.tensor_tensor(out=muA[:, b:b + 1],
                                            in0=mu[:, b:b + 1],
                                            in1=A[:, b:b + 1], op=AOP.mult)
                    nc.vector.tensor_tensor(out=B[:, b:b + 1],
                                            in0=bnb(l, b),
                                            in1=muA[:, b:b + 1],
                                            op=AOP.subtract)
                return A, B

            # ================================================== pipeline
            if phase_sel == "t1":
                spmm_pass(xtbl_t[:], P, out_slices(u1buf), "u1")
                sweep(u1buf, P, [(0, None, out_t, 0, True)])

            if phase_sel == "full":
                for k in ("S0", "Q0", "S1", "Q1", "S2", "Q2"):
                    st[(1, k)] = stpool.tile([P, NG], f32, name=f"st1{k}")
                    st[(2, k)] = stpool.tile([P, NG], f32, name=f"st2{k}")

                # -------- L1 spmm chain

                # -------- L1 transform (hop-outer so hop-0 work runs
                # during the u-passes; hop-2 gates only its own tail)
                def l1_hop(hop):
                    for grp in range(NG):
                        n0 = grp * GSZ
                        w = min(GSZ, SHARD - n0)
                        yT = hpool.tile([P, GSZ], bf16, name="yT",
                                        tag="yT")
                        if hop == 0:
                            nc.sync.dma_start(yT[:, :w], xt_t[:, n0:n0 + w])
                        else:
                            ybuf = y1buf if hop == 1 else y2buf
                            nc.sync.dma_start_transpose(yT[:, :w],
                                                        ybuf[n0:n0 + w, :])
                        ps1 = ps_tf.tile([P, GSZ], f32, name="tf1", tag="tf",
                                         space="PSUM")
                        nc.tensor.matmul(ps1[:, :w],
                                         lhsT=w0_sb[:, hop * P:(hop + 1) * P],
                                         rhs=yT[:, :w], start=True,
                                         stop=False)
                        which = 2 if hop == 0 else hop - 1
                        nc.tensor.matmul(ps1[:, :w], lhsT=brow(0, hop),
                                         rhs=load_row(which, n0, w, "rs"),
                                         start=False, stop=True)
                        nc.vector.reduce_sum(
                            out=st[(1, f"S{hop}")][:, grp:grp + 1],
                            in_=ps1[:, :w], axis=mybir.AxisListType.X)
                        cp2 = hpool.tile([P, GSZ], bf16, name="cpt2",
                                         tag="cpt")
                        nc.scalar.activation(out=cp2[:, :w], in_=ps1[:, :w],
                                             func=AF.Copy)
                        sq2 = wpool.tile([P, GSZ], f32, name="sqt2",
                                         tag="sqt")
                        nc.vector.tensor_tensor(out=sq2[:, :w],
                                                in0=cp2[:, :w],
                                                in1=cp2[:, :w], op=AOP.mult)
                        nc.vector.reduce_sum(
                            out=st[(1, f"Q{hop}")][:, grp:grp + 1],
                            in_=sq2[:, :w], axis=mybir.AxisListType.X)
                        nc.sync.dma_start(raw1[hop][:, n0:n0 + w],
                                          cp2[:, :w])
                # interleave: hop-0 fills the u1-RS wait; hop-1 overlaps
                # u2's gathers; hop-2 runs after u2's RS.
                spmm_pass(xtbl_t[:], P, out_slices(tbl2), "u1",
                          img128=dg2_sb)
                l1_hop(0)
                sweep(tbl2, P, [(0, dlocI_sb, y1buf, 0, False)])
                l1_hop(1)
                spmm_pass(tbl2[:], P, out_slices(y2buf), "u2")
                l1_hop(2)
                A1, B1 = bn_reduce_and_AB(
                    0, [(1, k) for k in ("S0", "Q0", "S1", "Q1", "S2", "Q2")])

                # -------- transform L2 / L3
                def transform_layer(l, A, B, final):
                    # block sources: ("fm", buf [P,SHARD]) feature-major or
                    # ("nm", buf [SHARD,P]) node-major (transpose-load)
                    if l == 1:
                        srcs = [("fm", raw1[0], 0), ("fm", raw1[1], 0),
                                ("fm", raw1[2], 0)]
                    else:
                        srcs = [("fm", raw20, 0), ("nm", rab[0], 0),
                                ("nm", rb2[0], 0)]
                    for grp in range(NG):
                        n0 = grp * GSZ
                        w = min(GSZ, SHARD - n0)
                        nq = w // P
                        hbt = []
                        for b in range(3):
                            kind_b, src, c0 = srcs[b]
                            raw = hpool.tile([P, GSZ], bf16, name="raw",
                                             tag=f"raw{b}")
                            if kind_b == "fm":
                                nc.sync.dma_start(raw[:, :w],
                                                  src[:, n0:n0 + w])
                            else:
                                nc.sync.dma_start_transpose(
                                    raw[:, :w],
                                    src[n0:n0 + w, c0:c0 + P])
                            h = hpool.tile([P, GSZ], bf16, name="hh",
                                           tag=f"h{b}")
                            nc.scalar.activation(out=h[:, :w],
                                                 in_=raw[:, :w],
                                                 func=AF.Relu,
                                                 bias=B[:, b:b + 1],
                                                 scale=A[:, b:b + 1])
                            hbt.append(h)
                        for j in range(3):
                            ps = ps_tf.tile([P, GSZ], f32, name="tfj",
                                            tag="tf", space="PSUM")
                            for b in range(3):
                                nc.tensor.matmul(ps[:, :w],
                                                 lhsT=wblk(l, j, b),
                                                 rhs=hbt[b][:, :w],
                                                 start=(b == 0), stop=False)
                            nc.tensor.matmul(ps[:, :w], lhsT=brow(l, j),
                                             rhs=load_row(2, n0, w, "ro"),
                                             start=False, stop=True)
                            if j == 0 and not final:
                                nc.vector.reduce_sum(
                                    out=st[(2, "S0")][:, grp:grp + 1],
                                    in_=ps[:, :w], axis=mybir.AxisListType.X)
                                cp = hpool.tile([P, GSZ], bf16, name="cpj",
                                                tag="cpt")
                                nc.scalar.activation(out=cp[:, :w],
                                                     in_=ps[:, :w],
                                                     func=AF.Copy)
                                sq = wpool.tile([P, GSZ], f32, name="sqj",
                                                tag="sqt")
                                nc.vector.tensor_tensor(
                                    out=sq[:, :w], in0=cp[:, :w],
                                    in1=cp[:, :w], op=AOP.mult)
                                nc.vector.reduce_sum(
                                    out=st[(2, "Q0")][:, grp:grp + 1],
                                    in_=sq[:, :w], axis=mybir.AxisListType.X)
                                nc.sync.dma_start(raw20[:, n0:n0 + w],
                                                  cp[:, :w])
                            elif j == 0 and final:
                                cpf = wpool.tile([P, GSZ], f32, name="cpf",
                                                 tag="cpf")
                                nc.scalar.activation(out=cpf[:, :w],
                                                     in_=ps[:, :w],
                                                     func=AF.Copy)
                                stf = wpool.tile([P, 4, P], f32, name="of",
                                                 tag="of")
                                for q in range(nq):
                                    pst = ps_tp.tile([P, P], f32, name="ptf",
                                                     tag="tpf", space="PSUM",
                                                     bufs=1)
                                    nc.tensor.transpose(
                                        pst[:], cpf[:, q * P:(q + 1) * P],
                                        identf_sb)
                                    nc.scalar.activation(out=stf[:, q, :],
                                                         in_=pst[:],
                                                         func=AF.Copy)
                                dv = out_t[n0:n0 + w, 0:P]
                                nc.sync.dma_start(
                                    dv.rearrange("(i p) f -> p i f", p=P),
                                    stf[:, :nq, :])
                            else:
                                # t_j -> transpose -> dloc1-scale -> table
                                dst = tblpair[l - 1]
                                cpb = wpool.tile([P, GSZ], bf16,
                                                 name="cpb", tag="cpb")
                                nc.vector.tensor_copy(out=cpb[:, :w],
                                                      in_=ps[:, :w])
                                stgt = stgpool.tile([P, 4, P], bf16,
                                                    name="tstg",
                                                    tag=f"tstg{j}")
                                for q in range(nq):
                                    pst = ps_tp.tile([P, P], bf16,
                                                     name="ptb", tag="tp",
                                                     space="PSUM")
                                    nc.tensor.transpose(
                                        pst[:], cpb[:, q * P:(q + 1) * P],
                                        ident_sb)
                                    tt = n0 // P + q
                                    nc.scalar.activation(
                                        out=stgt[:, q, :], in_=pst[:],
                                        func=AF.Copy,
                                        scale=dloc1_sb[:, tt:tt + 1])
                                dv = dst[n0:n0 + w,
                                         (j - 1) * P:j * P]
                                nc.sync.dma_start(
                                    dv.rearrange("(i p) f -> p i f", p=P),
                                    stgt[:, :nq, :])

                # L2
                transform_layer(1, A1, B1, final=False)
                spmm_pass(tblpair[0][:], 2 * P, out_slices(rab[0]), "v1")
                sweep(rab[0], 2 * P, [(P, None, tblu[0], 0, False)])
                stat_sweep(rab[0], 0, (2, "S1"), (2, "Q1"))
                spmm_pass(tblu[0][:], P, out_slices(rb2[0]), "v2")
                stat_sweep(rb2[0], 0, (2, "S2"), (2, "Q2"))
                A2, B2 = bn_reduce_and_AB(
                    1, [(2, k) for k in ("S0", "Q0", "S1", "Q1", "S2", "Q2")])

                # L3
                transform_layer(2, A2, B2, final=True)
                spmm_pass(tblpair[1][:], 2 * P, out_slices(rab[1]), "w1")
                sweep(rab[1], 2 * P, [(P, None, tblu[1], 0, False)])
                spmm_pass(tblu[1][:], P, out_slices(rb2[1]), "w2")
                sweep(rab[1], 2 * P, [(0, None, out_t, P, True)])
                sweep(rb2[1], P, [(0, None, out_t, 2 * P, True)])

    if not for_sim:
        _split_excess_waits(nc)
        mybir.codegen_inst_isa_subclasses(nc)
    return nc


_CACHE = {}


def kernel(x, edge_index, W0, b0, W1, b1, W2, b2, bn_gamma, bn_beta):
    x = np.asarray(x, np.float32)
    edge_index = np.asarray(edge_index)
    cores = _host_prep(x, edge_index)

    W0 = np.asarray(W0, np.float32)
    W1 = np.asarray(W1, np.float32)
    W2 = np.asarray(W2, np.float32)
    b0 = np.asarray(b0, np.float32)
    b1 = np.asarray(b1, np.float32)
    b2 = np.asarray(b2, np.float32)
    bn_g = np.asarray(bn_gamma, np.float32)
    bn_b = np.asarray(bn_beta, np.float32)

    in_maps = []
    for c in range(NC):
        cb, cr, cf = _pack_consts(cores[c], W0, b0, W1, b1, W2, b2,
                                  bn_g, bn_b)
        in_maps.append(dict(
            xtbl=cores[c]["xtbl"], xt=cores[c]["xt"], idx=cores[c]["idx16"],
            cb=cb, cr=cr, cf=cf))

    phase = os.environ.get("KPHASE", "full")
    if phase not in _CACHE:
        _CACHE[phase] = _build(phase)
    nc = _CACHE[phase]
    res = run_bass_kernel_spmd(nc, in_maps, core_ids=list(range(NC)),
                               trace=bool(os.environ.get("KERNEL_TRACE")))
    global last_result
    last_result = res
    out = np.concatenate([r["out"] for r in res.results], axis=0)
    return out[:N].astype(np.float32)


last_result = None


# revision 4
# speedup vs baseline: 1.1335x; 1.0718x over previous
"""MixHop GNN v2: source-stationary SpMM + ReduceScatter on 8 trn2 cores.

vs v1 (gather-from-replicated-table + AllGather):
 - Each core owns src shard [12544 rows]; every SpMM gathers ONLY from the
   local-shard table (int16 idx, no quartering) and produces a partial for
   ALL 784 dst tiles; a ReduceScatter(add) returns the local shard of the
   aggregate. RS is ~2-3x cheaper than AG in the cost model and is split
   into 2 phase-halves that overlap with compute.
 - Edge slots: per rank-of-98-tiles: 24 quads (4 tiles) + 1 pair; each tile
   has one 128-slot c1 chunk; each group has ONE shared 128-slot overflow
   chunk (c2). 125952 slots vs 150528 (v1) for the same edges.
 - dinv[dst] scaling folded into the PSUM drains (pre-RS; commutes with the
   sum), so RS outputs are directly h-ready or table-ready (dinv^2).
 - L2/L3 first hops share one pass: table [12544, 256] = [dinv*t1|dinv*t2],
   512B gather elements (no <512B DMA penalty), one sel serves both.
 - Transposes via HWDGE dma_start_transpose instead of PE+copy.
"""
import os
import numpy as np
import ml_dtypes

import concourse.bass as bass
import concourse.mybir as mybir
import concourse.tile as tile
from concourse import library_config
from concourse.bass_utils import run_bass_kernel_spmd

bf16 = mybir.dt.bfloat16
f32 = mybir.dt.float32
i16 = mybir.dt.int16
BF = ml_dtypes.bfloat16

N = 100000
NC = 8
P = 128
D = 128
SHARD = 12544
NPAD = SHARD * NC
RT = 98                  # tiles per rank
NQ = 24                  # quads per rank (plus 1 trailing pair)
PH1T = 48                # tiles in phase 1 (quads 0..11)
PH2T = 50                # tiles in phase 2 (quads 12..23 + pair)
GSZ = 512
NG = (SHARD + GSZ - 1) // GSZ   # 25 transform groups
EPS = 1e-5

# groups per rank: 24 quads + 1 pair; chunks: quads 5, pair 3
CPR = NQ * 5 + 3         # 123 chunks per rank
NSLOT = NC * CPR * P     # 125952
NC1 = 784                # c1 chunks total
NC2 = NC * (NQ + 1)      # 200 overflow chunks

AOP = mybir.AluOpType
AF = mybir.ActivationFunctionType

_SKIP_WAITSPLIT = (mybir.InstEventSemaphore,)


def _split_excess_waits(nc, keep=1):
    n = 0
    uid = [0]
    for fn in nc.m.functions:
        for blk in fn.blocks:
            insts = list(blk.instructions)
            out = []
            for inst in insts:
                si = inst.sync_info
                if (si is not None and si.on_wait and len(si.on_wait) > keep
                        and not isinstance(inst, _SKIP_WAITSPLIT)):
                    waits = list(si.on_wait)
                    extra, rest = waits[:-keep], waits[-keep:]
                    for w in extra:
                        uid[0] += 1
                        out.append(mybir.InstEventSemaphore(
                            name=f"evws_{uid[0]}",
                            engine=inst.engine,
                            ins=[], outs=[],
                            sync_info=mybir.SyncInfo(on_wait=[w], on_update=[]),
                        ))
                        n += 1
                    inst.sync_info = mybir.SyncInfo(
                        on_wait=rest, on_update=list(si.on_update or []))
                out.append(inst)
            if len(out) != len(insts):
                blk.instructions = out
    return n


# ------------------------------------------------------------------ batches
def batch_plan():
    """Static per-(phase, rank) batch structure.

    Returns list over phases of list over ranks of batches; each batch is a
    list of groups; each group is (kind, tiles) with kind in {"q","p"} and
    tiles = local tile indices (lt) in ascending order.
    """
    phases = []
    for ph in (0, 1):
        ranks = []
        for r in range(NC):
            if ph == 0:
                quads = [("q", [4 * q + i for i in range(4)])
                         for q in range(12)]
                batches = [quads[:6], quads[6:]]
            else:
                quads = [("q", [4 * q + i for i in range(4)])
                         for q in range(12, 24)]
                pair = ("p", [96, 97])
                batches = [quads[:6], quads[6:] + [pair]]
            ranks.append(batches)
        phases.append(ranks)
    return phases


# ---------------------------------------------------------------- host prep
def _host_prep(x, edge_index):
    row = edge_index[0].astype(np.int64)
    col = edge_index[1].astype(np.int64)
    deg = np.bincount(col, minlength=N).astype(np.float64)
    dinv = np.where(deg > 0, 1.0 / np.sqrt(np.maximum(deg, 1.0)), 0.0)
    s1 = dinv * np.bincount(col, weights=dinv[row], minlength=N)
    s2 = dinv * np.bincount(col, weights=dinv[row] * s1[row], minlength=N)

    dinv_pad = np.zeros(NPAD)
    dinv_pad[:N] = dinv
    x_pad = np.zeros((NPAD, D), np.float32)
    x_pad[:N] = x
    s1_pad = np.zeros(NPAD, np.float32)
    s1_pad[:N] = s1
    s2_pad = np.zeros(NPAD, np.float32)
    s2_pad[:N] = s2
    ones_pad = np.zeros(NPAD, np.float32)
    ones_pad[:N] = 1.0

    plan = batch_plan()
    cores = []
    for c in range(NC):
        lo = c * SHARD
        m = (row >= lo) & (row < lo + SHARD)
        r_l = (row[m] - lo).astype(np.int64)
        cc = col[m]
        t = cc >> 7
        order = np.argsort(t, kind="stable")
        r_l, cc, t = r_l[order], cc[order], t[order]
        cnt = np.bincount(t, minlength=784)
        starts = np.zeros(785, np.int64)
        starts[1:] = np.cumsum(cnt)

        idx = np.zeros(NSLOT, np.int16)
        ld1 = np.full((P, NC1), -1.0, np.float32)
        ld2a = np.full((P, NC2), -1.0, np.float32)
        ld2b = np.full((P, NC2), -1.0, np.float32)
        slot = 0
        c1i = 0
        c2i = 0
        for ph in (0, 1):
            for r in range(NC):
                for batch in plan[ph][r]:
                    for kind, lts in batch:
                        ov_idx = []
                        ov_rel = []
                        for j, lt in enumerate(lts):
                            gt = r * RT + lt
                            s, e = starts[gt], starts[gt + 1]
                            n1 = min(e - s, P)
                            idx[slot:slot + n1] = r_l[s:s + n1]
                            ld1[:n1, c1i] = (cc[s:s + n1] & 127)
                            slot += P
                            c1i += 1
                            if e - s > n1:
                                ov_idx.append(r_l[s + n1:e])
                                ov_rel.append((cc[s + n1:e] & 127) + P * j)
                        ov_idx = (np.concatenate(ov_idx) if ov_idx
                                  else np.zeros(0, np.int64))
                        ov_rel = (np.concatenate(ov_rel) if ov_rel
                                  else np.zeros(0, np.int64))
                        no = len(ov_idx)
                        assert no <= P, f"overflow {no} > 128"
                        idx[slot:slot + no] = ov_idx
                        ra = ov_rel.astype(np.float32)
                        ld2a[:no, c2i] = np.where(ov_rel < 256, ra, -1.0)
                        ld2b[:no, c2i] = np.where(ov_rel >= 256, ra - 256, -1.0)
                        slot += P
                        c2i += 1
        assert slot == NSLOT and c1i == NC1 and c2i == NC2

        # idx16: per batch segment, 16-wrap + 8x replicate
        idx16 = np.zeros((P, NSLOT // 16), np.int16)
        pos = 0
        for ph in (0, 1):
            for r in range(NC):
                for batch in plan[ph][r]:
                    nch = sum(len(g[1]) + 1 for g in batch)
                    nidx = nch * P
                    seg = idx[pos:pos + nidx]
                    blk = seg.reshape(-1, 16).T
                    idx16[:, pos // 16:(pos + nidx) // 16] = np.tile(blk, (8, 1))
                    pos += nidx

        dl = dinv_pad[lo:lo + SHARD]
        xtbl = (dl[:, None] * x_pad[lo:lo + SHARD]).astype(BF)
        xt = x_pad[lo:lo + SHARD].T.astype(BF)
        dloc1 = dl.reshape(RT, P).T.astype(np.float32)
        dloc2 = (dl ** 2).reshape(RT, P).T.astype(np.float32)
        dlI = np.where(dl > 0, 1.0 / np.maximum(dl, 1e-30), 0.0)
        dlocI = dlI.reshape(RT, P).T.astype(np.float32)
        dg1 = dinv_pad.reshape(784, P).T.astype(np.float32)
        dg2 = (dinv_pad ** 2).reshape(784, P).T.astype(np.float32)
        dgAB = np.empty((P, 784 * 2), np.float32)
        dgAB[:, 0::2] = dg1
        dgAB[:, 1::2] = dg2
        srow = np.concatenate([s1_pad[lo:lo + SHARD], s2_pad[lo:lo + SHARD]])
        onesr = ones_pad[lo:lo + SHARD]
        cores.append(dict(idx16=idx16, ld1=ld1, ld2a=ld2a, ld2b=ld2b,
                          xtbl=xtbl, xt=xt, dloc1=dloc1, dloc2=dloc2,
                          dlocI=dlocI, dg1=dg1, dgAB=dgAB,
                          srow=srow[None, :].astype(BF),
                          onesr=onesr[None, :].astype(BF)))
    return cores


def _pack_consts(core, W0, b0, W1, b1, W2, b2, bn_g, bn_b):
    iota1 = np.tile(np.arange(P, dtype=np.float32), (P, 1)).astype(BF)
    iota2 = np.tile(np.arange(2 * P, dtype=np.float32), (P, 1)).astype(BF)
    ident = np.eye(P, dtype=np.float32)
    w0c = np.concatenate([W0[j] for j in range(3)], axis=1)
    blocks = []
    for W in (W1, W2):
        for j in range(3):
            for b in range(3):
                blocks.append(W[j][b * P:(b + 1) * P, :])
    w12c = np.concatenate(blocks, axis=1)
    cb = np.concatenate([iota1, iota2, core["ld1"].astype(BF),
                         core["ld2a"].astype(BF), core["ld2b"].astype(BF),
                         ident.astype(BF), w0c.astype(BF), w12c.astype(BF)],
                        axis=1)
    br = np.concatenate([b0.reshape(1, -1), b1.reshape(1, -1),
                         b2.reshape(1, -1)], axis=1)
    cr = np.concatenate([br.astype(np.float32),
                         core["srow"].astype(np.float32),
                         core["onesr"].astype(np.float32)], axis=1).astype(BF)
    bng = np.stack([bn_g[l].reshape(3, P).T for l in range(2)], axis=0)
    bnb = np.stack([bn_b[l].reshape(3, P).T for l in range(2)], axis=0)
    cf = np.concatenate([core["dloc1"], core["dloc2"], core["dlocI"],
                         bng[0], bng[1], bnb[0], bnb[1], ident,
                         core["dg1"], core["dgAB"]], axis=1)
    return cb.astype(BF), cr, cf.astype(np.float32)


NCB = P + 2 * P + NC1 + 2 * NC2 + P + 3 * P + 18 * P
NCR = 9 * P + 2 * SHARD + SHARD
NCF = 3 * RT + 12 + P + 784 * 3


# ---------------------------------------------------------------- device
def _build(phase_sel="full", for_sim=False):
    plan = batch_plan()
    nc = bass.Bass(num_devices=NC)
    xtbl_t = nc.dram_tensor("xtbl", [SHARD, P], bf16, kind="ExternalInput")
    xt_t = nc.dram_tensor("xt", [P, SHARD], bf16, kind="ExternalInput")
    idx_t = nc.dram_tensor("idx", [P, NSLOT // 16], i16, kind="ExternalInput")
    cb_t = nc.dram_tensor("cb", [P, NCB], bf16, kind="ExternalInput")
    cr_t = nc.dram_tensor("cr", [1, NCR], bf16, kind="ExternalInput")
    cf_t = nc.dram_tensor("cf", [P, NCF], f32, kind="ExternalInput")
    out_t = nc.dram_tensor("out", [SHARD, 3 * P], f32, kind="ExternalOutput")

    with tile.TileContext(nc) as tc:
        with (
            tc.tile_pool(name="const", bufs=1) as cpool,
            tc.tile_pool(name="gath", bufs=2) as gpool,
            tc.tile_pool(name="sel", bufs=2) as spool,
            tc.tile_pool(name="stg", bufs=2) as stgpool,
            tc.tile_pool(name="work", bufs=3) as wpool,
            tc.tile_pool(name="hb", bufs=2) as hpool,
            tc.tile_pool(name="stat", bufs=1) as stpool,
            tc.tile_pool(name="psp", bufs=3, space="PSUM") as ps_sp,
            tc.tile_pool(name="ptf", bufs=2, space="PSUM") as ps_tf,
            tc.tile_pool(name="ptp", bufs=2, space="PSUM") as ps_tp,
            tc.tile_pool(name="dram", bufs=1, space="DRAM") as dpool,
        ):
            nc.gpsimd.load_library(library_config.mlp)

            idx_sb = cpool.tile([P, NSLOT // 16], i16)
            nc.sync.dma_start(idx_sb[:], idx_t[:])
            cb_sb = cpool.tile([P, NCB], bf16)
            nc.sync.dma_start(cb_sb[:], cb_t[:])
            bias_sb = cpool.tile([1, 9 * P], bf16)
            nc.sync.dma_start(bias_sb[:], cr_t[:1, :9 * P])
            cf_sb = cpool.tile([P, NCF], f32)
            nc.sync.dma_start(cf_sb[:], cf_t[:])

            o = 0
            iota1_sb = cb_sb[:, o:o + P]; o += P
            iota2_sb = cb_sb[:, o:o + 2 * P]; o += 2 * P
            ld1_sb = cb_sb[:, o:o + NC1]; o += NC1
            ld2a_sb = cb_sb[:, o:o + NC2]; o += NC2
            ld2b_sb = cb_sb[:, o:o + NC2]; o += NC2
            ident_sb = cb_sb[:, o:o + P]; o += P
            w0_sb = cb_sb[:, o:o + 3 * P]; o += 3 * P
            w12_sb = cb_sb[:, o:o + 18 * P]

            def wblk(l, j, b):
                i = ((l - 1) * 9 + j * 3 + b) * P
                return w12_sb[:, i:i + P]

            def brow(l, j):
                i = (l * 3 + j) * P
                return bias_sb[:1, i:i + P]

            def load_row(which, n0, w, tag):
                i = 9 * P + which * SHARD + n0
                rt_ = wpool.tile([1, GSZ], bf16, name="row", tag=tag)
                nc.sync.dma_start(rt_[:1, :w], cr_t[:1, i:i + w])
                return rt_[:1, :w]

            dloc1_sb = cf_sb[:, 0:RT]
            dloc2_sb = cf_sb[:, RT:2 * RT]
            dlocI_sb = cf_sb[:, 2 * RT:3 * RT]

            def bng(l, b):
                i = 3 * RT + l * 3 + b
                return cf_sb[:, i:i + 1]

            def bnb(l, b):
                i = 3 * RT + 6 + l * 3 + b
                return cf_sb[:, i:i + 1]

            identf_sb = cf_sb[:, 3 * RT + 12:3 * RT + 12 + P]
            _o2 = 3 * RT + 12 + P
            dg1_sb = cf_sb[:, _o2:_o2 + 784]
            dgAB_sb = cf_sb[:, _o2 + 784:_o2 + 784 * 3]
            dg2_sb = (dgAB_sb.rearrange("p (t two) -> p t two", two=2)
                      [:, :, 1])

            # DRAM buffers
            def dbuf(name, rows, colsw, dt=bf16):
                return dpool.tile([rows, colsw], dt, name=name)

            # partials: [*,128] for single passes, [*,256] for paired
            partR = [dbuf("pR1", NC * 6144, P), dbuf("pR2", NC * 6400, P)]
            partAB = [dbuf("pAB1", NC * 6144, 2 * P),
                      dbuf("pAB2", NC * 6400, 2 * P)]
            u1buf = dbuf("u1buf", SHARD, P)
            y1buf = dbuf("y1buf", SHARD, P)       # dinv*R1
            y2buf = dbuf("y2buf", SHARD, P)       # dinv*R2
            tbl2 = dbuf("tbl2", SHARD, P)         # dinv^2*R1
            tblpair = [dbuf(f"tblpair{l}", SHARD, 2 * P) for l in (0, 1)]
            rab = [dbuf(f"rab{l}", SHARD, 2 * P) for l in (0, 1)]
            tblu = [dbuf(f"tblu{l}", SHARD, P) for l in (0, 1)]
            rb2 = [dbuf(f"rb2_{l}", SHARD, P) for l in (0, 1)]
            raw1 = [dbuf(f"raw1_{b}", P, SHARD) for b in range(3)]
            raw20 = dbuf("raw2_0", P, SHARD)
            arin = [dpool.tile([P, 8], f32, name=f"arin{l}") for l in range(2)]
            arout = [dpool.tile([P, 8], f32, name=f"arout{l}",
                                addr_space="Shared") for l in range(2)]

            nregs = {n: nc.gpsimd.to_reg(n * P)
                     for n in (15, 16, 17)}

            # ------------------------------------------------ spmm pass
            def spmm_pass(tbl_ap, F, outs, uname, img128=None):
                """tbl_ap: [SHARD, F] gather table (DRAM). Produces the
                UNSCALED aggregate: partial -> per-phase ReduceScatter into
                outs = (phase0_out_ap, phase1_out_ap). dinv scalings happen
                in post-RS sweeps."""
                slot0 = [0]
                c1c = [0]
                c2c = [0]
                for ph in (0, 1):
                    rowbase = 0
                    pbuf = (partR if F == P else partAB)[ph]
                    for r in range(NC):
                        for bi, batch in enumerate(plan[ph][r]):
                            nt = sum(len(g[1]) for g in batch)
                            nch = nt + len(batch)
                            nidx = nch * P
                            graw = gpool.tile([P, 33 * 2 * P], bf16,
                                              name=f"g{uname}", tag="g")
                            g = (graw[:, :nch * F]
                                 .rearrange("p (c e) -> p c e", e=F))
                            seg = slot0[0] // 16
                            h1 = (nch + 1) // 2
                            for c0, c1 in ((0, h1), (h1, nch)):
                                nsub = c1 - c0
                                sg = seg + c0 * P // 16
                                nc.gpsimd.dma_gather(
                                    out_ap=g[:, c0:c1, :],
                                    in_ap=tbl_ap,
                                    idxs_ap=idx_sb[:, sg:sg +
                                                   nsub * P // 16],
                                    num_idxs=nsub * P,
                                    num_idxs_reg=nregs[nsub],
                                    elem_size=F,
                                    single_packet=False,
                                )
                            nc1b = nt
                            ngrp = len(batch)
                            sel1 = spool.tile([P, 26, P], bf16,
                                              name=f"s1{uname}", tag="s1")
                            nc.vector.tensor_tensor(
                                out=sel1[:, :nc1b, :],
                                in0=iota1_sb[:, None, :]
                                    .to_broadcast([P, nc1b, P]),
                                in1=ld1_sb[:, c1c[0]:c1c[0] + nc1b, None]
                                    .to_broadcast([P, nc1b, P]),
                                op=AOP.is_equal)
                            sel2a = spool.tile([P, 7, 2 * P], bf16,
                                               name=f"s2a{uname}", tag="s2a")
                            nc.vector.tensor_tensor(
                                out=sel2a[:, :ngrp, :],
                                in0=iota2_sb[:, None, :]
                                    .to_broadcast([P, ngrp, 2 * P]),
                                in1=ld2a_sb[:, c2c[0]:c2c[0] + ngrp, None]
                                    .to_broadcast([P, ngrp, 2 * P]),
                                op=AOP.is_equal)
                            nquad = sum(1 for k, _ in batch if k == "q")
                            sel2b = spool.tile([P, 7, 2 * P], bf16,
                                               name=f"s2b{uname}", tag="s2b")
                            if nquad:
                                nc.vector.tensor_tensor(
                                    out=sel2b[:, :nquad, :],
                                    in0=iota2_sb[:, None, :]
                                        .to_broadcast([P, nquad, 2 * P]),
                                    in1=ld2b_sb[:, c2c[0]:c2c[0] + nquad, None]
                                        .to_broadcast([P, nquad, 2 * P]),
                                    op=AOP.is_equal)
                            stg = stgpool.tile([P, 26 * 2 * P], bf16,
                                               name=f"st{uname}", tag="st")
                            gpos = 0
                            s1pos = 0
                            tpos = 0
                            ndr = 0
                            for gi, (kind, lts) in enumerate(batch):
                                ntl = len(lts)
                                c2pos = gpos + ntl
                                tpg = (2 * P) // F   # tiles per psum group
                                psb = None
                                for j, lt in enumerate(lts):
                                    if j % tpg == 0:
                                        psb = ps_sp.tile([P, 2 * P], f32,
                                                         name=f"ps{uname}",
                                                         tag="ps",
                                                         space="PSUM")
                                    k = j % tpg
                                    ps = psb[:, k * F:(k + 1) * F]
                                    nc.tensor.matmul(
                                        ps, lhsT=sel1[:, s1pos + j, :],
                                        rhs=g[:, gpos + j, :],
                                        start=True, stop=False)
                                    selc = sel2a if j < 2 else sel2b
                                    half = (j & 1) * P
                                    nc.tensor.matmul(
                                        ps,
                                        lhsT=selc[:, gi, half:half + P],
                                        rhs=g[:, c2pos, :],
                                        start=False, stop=True)
                                    if j % tpg == tpg - 1 or j == ntl - 1:
                                        # drain k+1 tiles, scaling by the
                                        # global dinv image (dinv^1 for
                                        # single passes; interleaved
                                        # dinv^1|dinv^2 for A|B halves of
                                        # paired passes)
                                        ntile = k + 1
                                        nls = ntile * F
                                        gt0 = r * RT + lts[j - k]
                                        dst3 = (stg[:, tpos * F:
                                                    tpos * F + nls]
                                                .rearrange("p (i f) -> p i f",
                                                           f=P))
                                        nhf = nls // P
                                        if F == P:
                                            im = (img128 if img128 is not None
                                                  else dg1_sb)
                                            img = im[:, gt0:gt0 + nhf, None]
                                        else:
                                            img = dgAB_sb[:, 2 * gt0:
                                                          2 * gt0 + nhf,
                                                          None]
                                        src3 = (psb[:, :nls]
                                                .rearrange("p (i f) -> p i f",
                                                           f=P))
                                        if F != P and ndr % 3 == 0:
                                            nc.vector.tensor_tensor(
                                                out=dst3, in0=src3,
                                                in1=img.to_broadcast(
                                                    [P, nhf, P]),
                                                op=AOP.mult)
                                        else:
                                            for ti in range(ntile):
                                                gt = gt0 + ti
                                                if F == P:
                                                    im0 = (img128 if img128
                                                           is not None
                                                           else dg1_sb)
                                                    nc.scalar.activation(
                                                        out=dst3[:, ti, :],
                                                        in_=src3[:, ti, :],
                                                        func=AF.Copy,
                                                        scale=im0[:,
                                                                  gt:gt + 1])
                                                else:
                                                    nc.scalar.activation(
                                                        out=dst3[:, 2 * ti,
                                                                 :],
                                                        in_=src3[:, 2 * ti,
                                                                 :],
                                                        func=AF.Copy,
                                                        scale=dg1_sb[
                                                            :, gt:gt + 1])
                                                    nc.scalar.activation(
                                                        out=dst3[:,
                                                                 2 * ti + 1,
                                                                 :],
                                                        in_=src3[:,
                                                                 2 * ti + 1,
                                                                 :],
                                                        func=AF.Copy,
                                                        scale=dg2_sb[
                                                            :, gt:gt + 1])
                                        ndr += 1
                                        tpos += ntile
                                gpos += ntl + 1
                                s1pos += ntl
                            # stage -> partial rows
                            rview = pbuf[rowbase:rowbase + nt * P, :]
                            nc.sync.dma_start(
                                rview.rearrange("(i p) f -> p i f", p=P),
                                stg[:, :nt * F]
                                .rearrange("p (i f) -> p i f", f=F))
                            rowbase += nt * P
                            slot0[0] += nidx
                            c1c[0] += nc1b
                            c2c[0] += ngrp
                    # RS for this phase
                    nc.gpsimd.collective_compute(
                        "ReduceScatter", AOP.add,
                        replica_groups=[list(range(NC))],
                        ins=[pbuf[:]], outs=[outs[ph]])

            def out_slices(buf):
                return (buf[0:6144, :], buf[6144:SHARD, :])

            # ------------------------------------------------ helpers
            def sweep(src, W, items):
                """Node-major dinv scaling: load src [SHARD, W] in 8-tile
                batches; for each (col0, dimg, dst_tensor, dcol0, f32out):
                dst rows = dimg-col(tile) * src[:, col0:col0+P]."""
                for t0 in range(0, RT, 4):
                    ntl = min(4, RT - t0)
                    ld = wpool.tile([P, 4, 2 * P], bf16, name="ssl",
                                    tag="ssl")
                    sv = src[t0 * P:(t0 + ntl) * P, :]
                    nc.sync.dma_start(
                        ld[:, :ntl, :W],
                        sv.rearrange("(i p) f -> p i f", p=P))
                    for col0, dimg, dst, dcol0, f32out in items:
                        dt_ = f32 if f32out else bf16
                        so = wpool.tile([P, 4, P], dt_, name="sso",
                                        tag=f"sso{1 if f32out else 0}")
                        if dimg is None:
                            nc.scalar.activation(
                                out=so[:, :ntl, :],
                                in_=ld[:, :ntl, col0:col0 + P],
                                func=AF.Copy)
                        else:
                            nc.vector.tensor_tensor(
                                out=so[:, :ntl, :],
                                in0=ld[:, :ntl, col0:col0 + P],
                                in1=dimg[:, t0:t0 + ntl, None]
                                    .to_broadcast([P, ntl, P]),
                                op=AOP.mult)
                        dv = dst[t0 * P:(t0 + ntl) * P, dcol0:dcol0 + P]
                        nc.sync.dma_start(
                            dv.rearrange("(i p) f -> p i f", p=P),
                            so[:, :ntl, :])

            st = {}

            def stat_sweep(buf, col0, keyS, keyQ):
                stS = st[keyS]; stQ = st[keyQ]
                for grp in range(NG):
                    n0 = grp * GSZ
                    w = min(GSZ, SHARD - n0)
                    par = grp % 2
                    tT = hpool.tile([P, GSZ], bf16, name="swT",
                                    tag="cpt" if par == 0 else "yT")
                    nc.sync.dma_start_transpose(
                        tT[:, :w], buf[n0:n0 + w, col0:col0 + P])
                    nc.vector.reduce_sum(out=stS[:, grp:grp + 1],
                                         in_=tT[:, :w],
                                         axis=mybir.AxisListType.X)
                    sq = wpool.tile([P, GSZ], f32, name="swq",
                                    tag="swq" if par == 0 else "sqt")
                    nc.vector.tensor_tensor(out=sq[:, :w], in0=tT[:, :w],
                                            in1=tT[:, :w], op=AOP.mult)
                    nc.vector.reduce_sum(out=stQ[:, grp:grp + 1],
                                         in_=sq[:, :w],
                                         axis=mybir.AxisListType.X)

            def bn_reduce_and_AB(l, keys):
                ar = wpool.tile([P, 8], f32, name=f"ar{l}", tag="ar")
                for b in range(3):
                    nc.vector.reduce_sum(out=ar[:, b:b + 1],
                                         in_=st[keys[2 * b]][:],
                                         axis=mybir.AxisListType.X)
                    nc.vector.reduce_sum(out=ar[:, 3 + b:4 + b],
                                         in_=st[keys[2 * b + 1]][:],
                                         axis=mybir.AxisListType.X)
                nc.sync.dma_start(arin[l][:], ar[:])
                nc.gpsimd.collective_compute(
                    "AllReduce", AOP.add, replica_groups=[list(range(NC))],
                    ins=[arin[l][:]], outs=[arout[l][:]])
                gg = wpool.tile([P, 8], f32, name=f"arg{l}", tag="ar")
                nc.sync.dma_start(gg[:], arout[l][:])
                A = stpool.tile([P, 3], f32, name=f"A{l}")
                B = stpool.tile([P, 3], f32, name=f"B{l}")
                mu = wpool.tile([P, 3], f32, name=f"mu{l}", tag="mu")
                va = wpool.tile([P, 3], f32, name=f"va{l}", tag="mu")
                nc.vector.tensor_scalar(out=mu[:], in0=gg[:, 0:3],
                                        scalar1=1.0 / N, scalar2=None,
                                        op0=AOP.mult)
                nc.vector.tensor_scalar(out=va[:], in0=gg[:, 3:6],
                                        scalar1=1.0 / N, scalar2=None,
                                        op0=AOP.mult)
                musq = wpool.tile([P, 3], f32, name=f"ms{l}", tag="mu")
                nc.vector.tensor_tensor(out=musq[:], in0=mu[:], in1=mu[:],
                                        op=AOP.mult)
                nc.vector.tensor_tensor(out=va[:], in0=va[:], in1=musq[:],
                                        op=AOP.subtract)
                ve = wpool.tile([P, 3], f32, name=f"ve{l}", tag="mu")
                nc.vector.tensor_scalar(out=ve[:], in0=va[:],
                                        scalar1=float(EPS), scalar2=None,
                                        op0=AOP.add)
                sq_ = wpool.tile([P, 3], f32, name=f"sv{l}", tag="mu")
                nc.scalar.activation(out=sq_[:], in_=ve[:], func=AF.Sqrt)
                rs = wpool.tile([P, 3], f32, name=f"rs{l}", tag="mu")
                nc.vector.reciprocal(out=rs[:], in_=sq_[:])
                muA = wpool.tile([P, 3], f32, name=f"ma{l}", tag="mu")
                for b in range(3):
                    nc.vector.tensor_tensor(out=A[:, b:b + 1],
                                            in0=rs[:, b:b + 1],
                                            in1=bng(l, b), op=AOP.mult)
                    nc.vector.tensor_tensor(out=muA[:, b:b + 1],
                                            in0=mu[:, b:b + 1],
                                            in1=A[:, b:b + 1], op=AOP.mult)
                    nc.vector.tensor_tensor(out=B[:, b:b + 1],
                                            in0=bnb(l, b),
                                            in1=muA[:, b:b + 1],
                                            op=AOP.subtract)
                return A, B

            # ================================================== pipeline
            if phase_sel == "t1":
                spmm_pass(xtbl_t[:], P, out_slices(u1buf), "u1")
                sweep(u1buf, P, [(0, None, out_t, 0, True)])

            if phase_sel == "full":
                for k in ("S0", "Q0", "S1", "Q1", "S2", "Q2"):
                    st[(1, k)] = stpool.tile([P, NG], f32, name=f"st1{k}")
                    st[(2, k)] = stpool.tile([P, NG], f32, name=f"st2{k}")

                # -------- L1 spmm chain

                # -------- L1 transform (hop-outer so hop-0 work runs
                # during the u-passes; hop-2 gates only its own tail)
                def l1_hop(hop):
                    for grp in range(NG):
                        n0 = grp * GSZ
                        w = min(GSZ, SHARD - n0)
                        yT = hpool.tile([P, GSZ], bf16, name="yT",
                                        tag="yT")
                        if hop == 0:
                            nc.sync.dma_start(yT[:, :w], xt_t[:, n0:n0 + w])
                        else:
                            ybuf = y1buf if hop == 1 else y2buf
                            nc.sync.dma_start_transpose(yT[:, :w],
                                                        ybuf[n0:n0 + w, :])
                        ps1 = ps_tf.tile([P, GSZ], f32, name="tf1", tag="tf",
                                         space="PSUM")
                        nc.tensor.matmul(ps1[:, :w],
                                         lhsT=w0_sb[:, hop * P:(hop + 1) * P],
                                         rhs=yT[:, :w], start=True,
                                         stop=False)
                        which = 2 if hop == 0 else hop - 1
                        nc.tensor.matmul(ps1[:, :w], lhsT=brow(0, hop),
                                         rhs=load_row(which, n0, w, "rs"),
                                         start=False, stop=True)
                        cp2 = hpool.tile([P, GSZ], bf16, name="cpt2",
                                         tag="cpt")
                        nc.scalar.activation(out=cp2[:, :w], in_=ps1[:, :w],
                                             func=AF.Copy)
                        nc.vector.reduce_sum(
                            out=st[(1, f"S{hop}")][:, grp:grp + 1],
                            in_=cp2[:, :w], axis=mybir.AxisListType.X)
                        sq2 = wpool.tile([P, GSZ], f32, name="sqt2",
                                         tag="sqt")
                        nc.vector.tensor_tensor(out=sq2[:, :w],
                                                in0=cp2[:, :w],
                                                in1=cp2[:, :w], op=AOP.mult)
                        nc.vector.reduce_sum(
                            out=st[(1, f"Q{hop}")][:, grp:grp + 1],
                            in_=sq2[:, :w], axis=mybir.AxisListType.X)
                        nc.sync.dma_start(raw1[hop][:, n0:n0 + w],
                                          cp2[:, :w])
                # interleave: hop-0 fills the u1-RS wait; hop-1 overlaps
                # u2's gathers; hop-2 runs after u2's RS.
                spmm_pass(xtbl_t[:], P, out_slices(tbl2), "u1",
                          img128=dg2_sb)
                l1_hop(0)
                sweep(tbl2, P, [(0, dlocI_sb, y1buf, 0, False)])
                l1_hop(1)
                spmm_pass(tbl2[:], P, out_slices(y2buf), "u2")
                l1_hop(2)
                A1, B1 = bn_reduce_and_AB(
                    0, [(1, k) for k in ("S0", "Q0", "S1", "Q1", "S2", "Q2")])

                # -------- transform L2 / L3
                def transform_layer(l, A, B, final):
                    # block sources: ("fm", buf [P,SHARD]) feature-major or
                    # ("nm", buf [SHARD,P]) node-major (transpose-load)
                    if l == 1:
                        srcs = [("fm", raw1[0], 0), ("fm", raw1[1], 0),
                                ("fm", raw1[2], 0)]
                    else:
                        srcs = [("fm", raw20, 0), ("nm", rab[0], 0),
                                ("nm", rb2[0], 0)]
                    for grp in range(NG):
                        n0 = grp * GSZ
                        w = min(GSZ, SHARD - n0)
                        nq = w // P
                        hbt = []
                        for b in range(3):
                            kind_b, src, c0 = srcs[b]
                            raw = hpool.tile([P, GSZ], bf16, name="raw",
                                             tag=f"raw{b}")
                            if kind_b == "fm":
                                nc.sync.dma_start(raw[:, :w],
                                                  src[:, n0:n0 + w])
                            else:
                                nc.sync.dma_start_transpose(
                                    raw[:, :w],
                                    src[n0:n0 + w, c0:c0 + P])
                            h = hpool.tile([P, GSZ], bf16, name="hh",
                                           tag=f"h{b}")
                            nc.scalar.activation(out=h[:, :w],
                                                 in_=raw[:, :w],
                                                 func=AF.Relu,
                                                 bias=B[:, b:b + 1],
                                                 scale=A[:, b:b + 1])
                            hbt.append(h)
                        for j in range(3):
                            ps = ps_tf.tile([P, GSZ], f32, name="tfj",
                                            tag="tf", space="PSUM")
                            for b in range(3):
                                nc.tensor.matmul(ps[:, :w],
                                                 lhsT=wblk(l, j, b),
                                                 rhs=hbt[b][:, :w],
                                                 start=(b == 0), stop=False)
                            nc.tensor.matmul(ps[:, :w], lhsT=brow(l, j),
                                             rhs=load_row(2, n0, w, "ro"),
                                             start=False, stop=True)
                            if j == 0 and not final:
                                cp = hpool.tile([P, GSZ], bf16, name="cpj",
                                                tag="cpt")
                                nc.scalar.activation(out=cp[:, :w],
                                                     in_=ps[:, :w],
                                                     func=AF.Copy)
                                nc.vector.reduce_sum(
                                    out=st[(2, "S0")][:, grp:grp + 1],
                                    in_=cp[:, :w], axis=mybir.AxisListType.X)
                                sq = wpool.tile([P, GSZ], f32, name="sqj",
                                                tag="sqt")
                                nc.vector.tensor_tensor(
                                    out=sq[:, :w], in0=cp[:, :w],
                                    in1=cp[:, :w], op=AOP.mult)
                                nc.vector.reduce_sum(
                                    out=st[(2, "Q0")][:, grp:grp + 1],
                                    in_=sq[:, :w], axis=mybir.AxisListType.X)
                                nc.sync.dma_start(raw20[:, n0:n0 + w],
                                                  cp[:, :w])
                            elif j == 0 and final:
                                cpf = wpool.tile([P, GSZ], f32, name="cpf",
                                                 tag="cpf")
                                nc.scalar.activation(out=cpf[:, :w],
                                                     in_=ps[:, :w],
                                                     func=AF.Copy)
                                stf = wpool.tile([P, 4, P], f32, name="of",
                                                 tag="of")
                                for q in range(nq):
                                    pst = ps_tp.tile([P, P], f32, name="ptf",
                                                     tag="tpf", space="PSUM",
                                                     bufs=1)
                                    nc.tensor.transpose(
                                        pst[:], cpf[:, q * P:(q + 1) * P],
                                        identf_sb)
                                    nc.scalar.activation(out=stf[:, q, :],
                                                         in_=pst[:],
                                                         func=AF.Copy)
                                dv = out_t[n0:n0 + w, 0:P]
                                nc.sync.dma_start(
                                    dv.rearrange("(i p) f -> p i f", p=P),
                                    stf[:, :nq, :])
                            else:
                                # t_j -> transpose -> dloc1-scale -> table
                                dst = tblpair[l - 1]
                                cpb = wpool.tile([P, GSZ], bf16,
                                                 name="cpb", tag="cpb")
                                nc.vector.tensor_copy(out=cpb[:, :w],
                                                      in_=ps[:, :w])
                                stgt = stgpool.tile([P, 4, P], bf16,
                                                    name="tstg",
                                                    tag=f"tstg{j}")
                                for q in range(nq):
                                    pst = ps_tp.tile([P, P], bf16,
                                                     name="ptb", tag="tp",
                                                     space="PSUM")
                                    nc.tensor.transpose(
                                        pst[:], cpb[:, q * P:(q + 1) * P],
                                        ident_sb)
                                    tt = n0 // P + q
                                    nc.scalar.activation(
                                        out=stgt[:, q, :], in_=pst[:],
                                        func=AF.Copy,
                                        scale=dloc1_sb[:, tt:tt + 1])
                                dv = dst[n0:n0 + w,
                                         (j - 1) * P:j * P]
                                nc.sync.dma_start(
                                    dv.rearrange("(i p) f -> p i f", p=P),
                                    stgt[:, :nq, :])

                # L2
                transform_layer(1, A1, B1, final=False)
                spmm_pass(tblpair[0][:], 2 * P, out_slices(rab[0]), "v1")
                sweep(rab[0], 2 * P, [(P, None, tblu[0], 0, False)])
                stat_sweep(rab[0], 0, (2, "S1"), (2, "Q1"))
                spmm_pass(tblu[0][:], P, out_slices(rb2[0]), "v2")
                stat_sweep(rb2[0], 0, (2, "S2"), (2, "Q2"))
                A2, B2 = bn_reduce_and_AB(
                    1, [(2, k) for k in ("S0", "Q0", "S1", "Q1", "S2", "Q2")])

                # L3
                transform_layer(2, A2, B2, final=True)
                spmm_pass(tblpair[1][:], 2 * P, out_slices(rab[1]), "w1")
                sweep(rab[1], 2 * P, [(P, None, tblu[1], 0, False)])
                spmm_pass(tblu[1][:], P, out_slices(rb2[1]), "w2")
                sweep(rab[1], 2 * P, [(0, None, out_t, P, True)])
                sweep(rb2[1], P, [(0, None, out_t, 2 * P, True)])

    if not for_sim:
        _split_excess_waits(nc)
        mybir.codegen_inst_isa_subclasses(nc)
    return nc


_CACHE = {}


def kernel(x, edge_index, W0, b0, W1, b1, W2, b2, bn_gamma, bn_beta):
    x = np.asarray(x, np.float32)
    edge_index = np.asarray(edge_index)
    cores = _host_prep(x, edge_index)

    W0 = np.asarray(W0, np.float32)
    W1 = np.asarray(W1, np.float32)
    W2 = np.asarray(W2, np.float32)
    b0 = np.asarray(b0, np.float32)
    b1 = np.asarray(b1, np.float32)
    b2 = np.asarray(b2, np.float32)
    bn_g = np.asarray(bn_gamma, np.float32)
    bn_b = np.asarray(bn_beta, np.float32)

    in_maps = []
    for c in range(NC):
        cb, cr, cf = _pack_consts(cores[c], W0, b0, W1, b1, W2, b2,
                                  bn_g, bn_b)
        in_maps.append(dict(
            xtbl=cores[c]["xtbl"], xt=cores[c]["xt"], idx=cores[c]["idx16"],
            cb=cb, cr=cr, cf=cf))

    phase = os.environ.get("KPHASE", "full")
    if phase not in _CACHE:
        _CACHE[phase] = _build(phase)
    nc = _CACHE[phase]
    res = run_bass_kernel_spmd(nc, in_maps, core_ids=list(range(NC)),
                               trace=bool(os.environ.get("KERNEL_TRACE")))
    global last_result
    last_result = res
    out = np.concatenate([r["out"] for r in res.results], axis=0)
    return out[:N].astype(np.float32)


last_result = None


# revision 5
# speedup vs baseline: 1.1661x; 1.0288x over previous
"""MixHop GNN v2: source-stationary SpMM + ReduceScatter on 8 trn2 cores.

vs v1 (gather-from-replicated-table + AllGather):
 - Each core owns src shard [12544 rows]; every SpMM gathers ONLY from the
   local-shard table (int16 idx, no quartering) and produces a partial for
   ALL 784 dst tiles; a ReduceScatter(add) returns the local shard of the
   aggregate. RS is ~2-3x cheaper than AG in the cost model and is split
   into 2 phase-halves that overlap with compute.
 - Edge slots: per rank-of-98-tiles: 24 quads (4 tiles) + 1 pair; each tile
   has one 128-slot c1 chunk; each group has ONE shared 128-slot overflow
   chunk (c2). 125952 slots vs 150528 (v1) for the same edges.
 - dinv[dst] scaling folded into the PSUM drains (pre-RS; commutes with the
   sum), so RS outputs are directly h-ready or table-ready (dinv^2).
 - L2/L3 first hops share one pass: table [12544, 256] = [dinv*t1|dinv*t2],
   512B gather elements (no <512B DMA penalty), one sel serves both.
 - Transposes via HWDGE dma_start_transpose instead of PE+copy.
"""
import os
import numpy as np
import ml_dtypes

import concourse.bass as bass
import concourse.mybir as mybir
import concourse.tile as tile
from concourse import library_config
from concourse.bass_utils import run_bass_kernel_spmd

bf16 = mybir.dt.bfloat16
f32 = mybir.dt.float32
i16 = mybir.dt.int16
BF = ml_dtypes.bfloat16

N = 100000
NC = 8
P = 128
D = 128
SHARD = 12544
NPAD = SHARD * NC
RT = 98                  # tiles per rank
NQ = 24                  # quads per rank (plus 1 trailing pair)
PH1T = 48                # tiles in phase 1 (quads 0..11)
PH2T = 50                # tiles in phase 2 (quads 12..23 + pair)
GSZ = 512
NG = (SHARD + GSZ - 1) // GSZ   # 25 transform groups
EPS = 1e-5

# groups per rank: 24 quads + 1 pair; chunks: quads 5, pair 3
CPR = NQ * 5 + 3         # 123 chunks per rank
NSLOT = NC * CPR * P     # 125952
NC1 = 784                # c1 chunks total
NC2 = NC * (NQ + 1)      # 200 overflow chunks

AOP = mybir.AluOpType
AF = mybir.ActivationFunctionType

_SKIP_WAITSPLIT = (mybir.InstEventSemaphore,)


def _split_excess_waits(nc, keep=1):
    n = 0
    uid = [0]
    for fn in nc.m.functions:
        for blk in fn.blocks:
            insts = list(blk.instructions)
            out = []
            for inst in insts:
                si = inst.sync_info
                if (si is not None and si.on_wait and len(si.on_wait) > keep
                        and not isinstance(inst, _SKIP_WAITSPLIT)):
                    waits = list(si.on_wait)
                    extra, rest = waits[:-keep], waits[-keep:]
                    for w in extra:
                        uid[0] += 1
                        out.append(mybir.InstEventSemaphore(
                            name=f"evws_{uid[0]}",
                            engine=inst.engine,
                            ins=[], outs=[],
                            sync_info=mybir.SyncInfo(on_wait=[w], on_update=[]),
                        ))
                        n += 1
                    inst.sync_info = mybir.SyncInfo(
                        on_wait=rest, on_update=list(si.on_update or []))
                out.append(inst)
            if len(out) != len(insts):
                blk.instructions = out
    return n


# ------------------------------------------------------------------ batches
def batch_plan():
    """Static per-(phase, rank) batch structure.

    Returns list over phases of list over ranks of batches; each batch is a
    list of groups; each group is (kind, tiles) with kind in {"q","p"} and
    tiles = local tile indices (lt) in ascending order.
    """
    phases = []
    for ph in (0, 1):
        ranks = []
        for r in range(NC):
            if ph == 0:
                quads = [("q", [4 * q + i for i in range(4)])
                         for q in range(12)]
                batches = [quads[:6], quads[6:]]
            else:
                quads = [("q", [4 * q + i for i in range(4)])
                         for q in range(12, 24)]
                pair = ("p", [96, 97])
                batches = [quads[:6], quads[6:] + [pair]]
            ranks.append(batches)
        phases.append(ranks)
    return phases


# ---------------------------------------------------------------- host prep
def _host_prep(x, edge_index):
    row = edge_index[0].astype(np.int64)
    col = edge_index[1].astype(np.int64)
    deg = np.bincount(col, minlength=N).astype(np.float64)
    dinv = np.where(deg > 0, 1.0 / np.sqrt(np.maximum(deg, 1.0)), 0.0)
    s1 = dinv * np.bincount(col, weights=dinv[row], minlength=N)
    s2 = dinv * np.bincount(col, weights=dinv[row] * s1[row], minlength=N)

    dinv_pad = np.zeros(NPAD)
    dinv_pad[:N] = dinv
    x_pad = np.zeros((NPAD, D), np.float32)
    x_pad[:N] = x
    s1_pad = np.zeros(NPAD, np.float32)
    s1_pad[:N] = s1
    s2_pad = np.zeros(NPAD, np.float32)
    s2_pad[:N] = s2
    ones_pad = np.zeros(NPAD, np.float32)
    ones_pad[:N] = 1.0

    plan = batch_plan()
    cores = []
    for c in range(NC):
        lo = c * SHARD
        m = (row >= lo) & (row < lo + SHARD)
        r_l = (row[m] - lo).astype(np.int64)
        cc = col[m]
        t = cc >> 7
        order = np.argsort(t, kind="stable")
        r_l, cc, t = r_l[order], cc[order], t[order]
        cnt = np.bincount(t, minlength=784)
        starts = np.zeros(785, np.int64)
        starts[1:] = np.cumsum(cnt)

        idx = np.zeros(NSLOT, np.int16)
        ld1 = np.full((P, NC1), -1.0, np.float32)
        ld2a = np.full((P, NC2), -1.0, np.float32)
        ld2b = np.full((P, NC2), -1.0, np.float32)
        slot = 0
        c1i = 0
        c2i = 0
        for ph in (0, 1):
            for r in range(NC):
                for batch in plan[ph][r]:
                    for kind, lts in batch:
                        ov_idx = []
                        ov_rel = []
                        for j, lt in enumerate(lts):
                            gt = r * RT + lt
                            s, e = starts[gt], starts[gt + 1]
                            n1 = min(e - s, P)
                            idx[slot:slot + n1] = r_l[s:s + n1]
                            ld1[:n1, c1i] = (cc[s:s + n1] & 127)
                            slot += P
                            c1i += 1
                            if e - s > n1:
                                ov_idx.append(r_l[s + n1:e])
                                ov_rel.append((cc[s + n1:e] & 127) + P * j)
                        ov_idx = (np.concatenate(ov_idx) if ov_idx
                                  else np.zeros(0, np.int64))
                        ov_rel = (np.concatenate(ov_rel) if ov_rel
                                  else np.zeros(0, np.int64))
                        no = len(ov_idx)
                        assert no <= P, f"overflow {no} > 128"
                        idx[slot:slot + no] = ov_idx
                        ra = ov_rel.astype(np.float32)
                        ld2a[:no, c2i] = np.where(ov_rel < 256, ra, -1.0)
                        ld2b[:no, c2i] = np.where(ov_rel >= 256, ra - 256, -1.0)
                        slot += P
                        c2i += 1
        assert slot == NSLOT and c1i == NC1 and c2i == NC2

        # idx16: per batch segment, 16-wrap + 8x replicate
        idx16 = np.zeros((P, NSLOT // 16), np.int16)
        pos = 0
        for ph in (0, 1):
            for r in range(NC):
                for batch in plan[ph][r]:
                    nch = sum(len(g[1]) + 1 for g in batch)
                    nidx = nch * P
                    seg = idx[pos:pos + nidx]
                    blk = seg.reshape(-1, 16).T
                    idx16[:, pos // 16:(pos + nidx) // 16] = np.tile(blk, (8, 1))
                    pos += nidx

        dl = dinv_pad[lo:lo + SHARD]
        xtbl = (dl[:, None] * x_pad[lo:lo + SHARD]).astype(BF)
        xt = x_pad[lo:lo + SHARD].T.astype(BF)
        dloc1 = dl.reshape(RT, P).T.astype(np.float32)
        dloc2 = (dl ** 2).reshape(RT, P).T.astype(np.float32)
        dlI = np.where(dl > 0, 1.0 / np.maximum(dl, 1e-30), 0.0)
        dlocI = dlI.reshape(RT, P).T.astype(np.float32)
        dg1 = dinv_pad.reshape(784, P).T.astype(np.float32)
        dg2 = (dinv_pad ** 2).reshape(784, P).T.astype(np.float32)
        dgAB = np.empty((P, 784 * 2), np.float32)
        dgAB[:, 0::2] = dg1
        dgAB[:, 1::2] = dg2
        srow = np.concatenate([s1_pad[lo:lo + SHARD], s2_pad[lo:lo + SHARD]])
        onesr = ones_pad[lo:lo + SHARD]
        cores.append(dict(idx16=idx16, ld1=ld1, ld2a=ld2a, ld2b=ld2b,
                          xtbl=xtbl, xt=xt, dloc1=dloc1, dloc2=dloc2,
                          dlocI=dlocI, dg1=dg1, dgAB=dgAB,
                          srow=srow[None, :].astype(BF),
                          onesr=onesr[None, :].astype(BF)))
    return cores


def _pack_consts(core, W0, b0, W1, b1, W2, b2, bn_g, bn_b):
    iota1 = np.tile(np.arange(P, dtype=np.float32), (P, 1)).astype(BF)
    iota2 = np.tile(np.arange(2 * P, dtype=np.float32), (P, 1)).astype(BF)
    ident = np.eye(P, dtype=np.float32)
    w0c = np.concatenate([W0[j] for j in range(3)], axis=1)
    blocks = []
    for W in (W1, W2):
        for j in range(3):
            for b in range(3):
                blocks.append(W[j][b * P:(b + 1) * P, :])
    w12c = np.concatenate(blocks, axis=1)
    cb = np.concatenate([iota1, iota2,
                         np.repeat(core["ld1"], 2, axis=1).astype(BF),
                         np.repeat(core["ld2a"], 2, axis=1).astype(BF),
                         np.repeat(core["ld2b"], 2, axis=1).astype(BF),
                         ident.astype(BF), w0c.astype(BF), w12c.astype(BF)],
                        axis=1)
    br = np.concatenate([b0.reshape(1, -1), b1.reshape(1, -1),
                         b2.reshape(1, -1)], axis=1)
    cr = np.concatenate([br.astype(np.float32),
                         core["srow"].astype(np.float32),
                         core["onesr"].astype(np.float32)], axis=1).astype(BF)
    bng = np.stack([bn_g[l].reshape(3, P).T for l in range(2)], axis=0)
    bnb = np.stack([bn_b[l].reshape(3, P).T for l in range(2)], axis=0)
    cf = np.concatenate([core["dloc1"], core["dloc2"], core["dlocI"],
                         bng[0], bng[1], bnb[0], bnb[1], ident,
                         core["dg1"], core["dgAB"]], axis=1)
    return cb.astype(BF), cr, cf.astype(np.float32)


NCB = P + 2 * P + 2 * NC1 + 4 * NC2 + P + 3 * P + 18 * P
NCR = 9 * P + 2 * SHARD + SHARD
NCF = 3 * RT + 12 + P + 784 * 3


# ---------------------------------------------------------------- device
def _build(phase_sel="full", for_sim=False):
    plan = batch_plan()
    nc = bass.Bass(num_devices=NC)
    xtbl_t = nc.dram_tensor("xtbl", [SHARD, P], bf16, kind="ExternalInput")
    xt_t = nc.dram_tensor("xt", [P, SHARD], bf16, kind="ExternalInput")
    idx_t = nc.dram_tensor("idx", [P, NSLOT // 16], i16, kind="ExternalInput")
    cb_t = nc.dram_tensor("cb", [P, NCB], bf16, kind="ExternalInput")
    cr_t = nc.dram_tensor("cr", [1, NCR], bf16, kind="ExternalInput")
    cf_t = nc.dram_tensor("cf", [P, NCF], f32, kind="ExternalInput")
    out_t = nc.dram_tensor("out", [SHARD, 3 * P], f32, kind="ExternalOutput")

    with tile.TileContext(nc) as tc:
        with (
            tc.tile_pool(name="const", bufs=1) as cpool,
            tc.tile_pool(name="gath", bufs=2) as gpool,
            tc.tile_pool(name="sel", bufs=2) as spool,
            tc.tile_pool(name="stg", bufs=2) as stgpool,
            tc.tile_pool(name="work", bufs=3) as wpool,
            tc.tile_pool(name="hb", bufs=2) as hpool,
            tc.tile_pool(name="stat", bufs=1) as stpool,
            tc.tile_pool(name="psp", bufs=3, space="PSUM") as ps_sp,
            tc.tile_pool(name="ptf", bufs=2, space="PSUM") as ps_tf,
            tc.tile_pool(name="ptp", bufs=2, space="PSUM") as ps_tp,
            tc.tile_pool(name="dram", bufs=1, space="DRAM") as dpool,
        ):
            nc.gpsimd.load_library(library_config.mlp)

            idx_sb = cpool.tile([P, NSLOT // 16], i16)
            nc.sync.dma_start(idx_sb[:], idx_t[:])
            cb_sb = cpool.tile([P, NCB], bf16)
            nc.sync.dma_start(cb_sb[:], cb_t[:])
            bias_sb = cpool.tile([1, 9 * P], bf16)
            nc.sync.dma_start(bias_sb[:], cr_t[:1, :9 * P])
            cf_sb = cpool.tile([P, NCF], f32)
            nc.sync.dma_start(cf_sb[:], cf_t[:])

            o = 0
            iota1_sb = cb_sb[:, o:o + P]; o += P
            iota2_sb = cb_sb[:, o:o + 2 * P]; o += 2 * P
            ld1_sb = cb_sb[:, o:o + 2 * NC1]; o += 2 * NC1
            ld2a_sb = cb_sb[:, o:o + 2 * NC2]; o += 2 * NC2
            ld2b_sb = cb_sb[:, o:o + 2 * NC2]; o += 2 * NC2
            ident_sb = cb_sb[:, o:o + P]; o += P
            w0_sb = cb_sb[:, o:o + 3 * P]; o += 3 * P
            w12_sb = cb_sb[:, o:o + 18 * P]

            def wblk(l, j, b):
                i = ((l - 1) * 9 + j * 3 + b) * P
                return w12_sb[:, i:i + P]

            def brow(l, j):
                i = (l * 3 + j) * P
                return bias_sb[:1, i:i + P]

            def load_row(which, n0, w, tag):
                i = 9 * P + which * SHARD + n0
                rt_ = wpool.tile([1, GSZ], bf16, name="row", tag=tag)
                nc.sync.dma_start(rt_[:1, :w], cr_t[:1, i:i + w])
                return rt_[:1, :w]

            dloc1_sb = cf_sb[:, 0:RT]
            dloc2_sb = cf_sb[:, RT:2 * RT]
            dlocI_sb = cf_sb[:, 2 * RT:3 * RT]

            def bng(l, b):
                i = 3 * RT + l * 3 + b
                return cf_sb[:, i:i + 1]

            def bnb(l, b):
                i = 3 * RT + 6 + l * 3 + b
                return cf_sb[:, i:i + 1]

            identf_sb = cf_sb[:, 3 * RT + 12:3 * RT + 12 + P]
            _o2 = 3 * RT + 12 + P
            dg1_sb = cf_sb[:, _o2:_o2 + 784]
            dgAB_sb = cf_sb[:, _o2 + 784:_o2 + 784 * 3]
            dg2_sb = (dgAB_sb.rearrange("p (t two) -> p t two", two=2)
                      [:, :, 1])

            # DRAM buffers
            def dbuf(name, rows, colsw, dt=bf16):
                return dpool.tile([rows, colsw], dt, name=name)

            # partials: [*,128] for single passes, [*,256] for paired
            partR = [dbuf("pR1", NC * 6144, P), dbuf("pR2", NC * 6400, P)]
            partAB = [dbuf("pAB1", NC * 6144, 2 * P),
                      dbuf("pAB2", NC * 6400, 2 * P)]
            u1buf = dbuf("u1buf", SHARD, P)
            y1buf = dbuf("y1buf", SHARD, P)       # dinv*R1
            y2buf = dbuf("y2buf", SHARD, P)       # dinv*R2
            tbl2 = dbuf("tbl2", SHARD, P)         # dinv^2*R1
            tblpair = [dbuf(f"tblpair{l}", SHARD, 2 * P) for l in (0, 1)]
            rab = [dbuf(f"rab{l}", SHARD, 2 * P) for l in (0, 1)]
            tblu = [dbuf(f"tblu{l}", SHARD, P) for l in (0, 1)]
            rb2 = [dbuf(f"rb2_{l}", SHARD, P) for l in (0, 1)]
            raw1 = [dbuf(f"raw1_{b}", P, SHARD) for b in range(3)]
            raw20 = dbuf("raw2_0", P, SHARD)
            arin = [dpool.tile([P, 8], f32, name=f"arin{l}") for l in range(2)]
            arout = [dpool.tile([P, 8], f32, name=f"arout{l}",
                                addr_space="Shared") for l in range(2)]

            nregs = {n: nc.gpsimd.to_reg(n * P)
                     for n in (15, 16, 17)}

            # ------------------------------------------------ spmm pass
            def spmm_pass(tbl_ap, F, outs, uname, img128=None):
                """tbl_ap: [SHARD, F] gather table (DRAM). Produces the
                UNSCALED aggregate: partial -> per-phase ReduceScatter into
                outs = (phase0_out_ap, phase1_out_ap). dinv scalings happen
                in post-RS sweeps."""
                slot0 = [0]
                c1c = [0]
                c2c = [0]
                for ph in (0, 1):
                    rowbase = 0
                    pbuf = (partR if F == P else partAB)[ph]
                    for r in range(NC):
                        for bi, batch in enumerate(plan[ph][r]):
                            nt = sum(len(g[1]) for g in batch)
                            nch = nt + len(batch)
                            nidx = nch * P
                            graw = gpool.tile([P, 33 * 2 * P], bf16,
                                              name=f"g{uname}", tag="g")
                            g = (graw[:, :nch * F]
                                 .rearrange("p (c e) -> p c e", e=F))
                            seg = slot0[0] // 16
                            h1 = (nch + 1) // 2
                            for c0, c1 in ((0, h1), (h1, nch)):
                                nsub = c1 - c0
                                sg = seg + c0 * P // 16
                                nc.gpsimd.dma_gather(
                                    out_ap=g[:, c0:c1, :],
                                    in_ap=tbl_ap,
                                    idxs_ap=idx_sb[:, sg:sg +
                                                   nsub * P // 16],
                                    num_idxs=nsub * P,
                                    num_idxs_reg=nregs[nsub],
                                    elem_size=F,
                                    single_packet=False,
                                )
                            nc1b = nt
                            ngrp = len(batch)
                            sel1 = spool.tile([P, 26, P], bf16,
                                              name=f"s1{uname}", tag="s1")
                            nc.vector.tensor_tensor(
                                out=sel1[:, :nc1b, :]
                                    .rearrange("p c (h t) -> p c h t", t=2),
                                in0=iota1_sb
                                    .rearrange("p (h t) -> p h t", t=2)
                                    [:, None, :, :]
                                    .to_broadcast([P, nc1b, P // 2, 2]),
                                in1=ld1_sb
                                    .rearrange("p (c t) -> p c t", t=2)
                                    [:, c1c[0]:c1c[0] + nc1b, None, :]
                                    .to_broadcast([P, nc1b, P // 2, 2]),
                                op=AOP.is_equal)
                            sel2a = spool.tile([P, 7, 2 * P], bf16,
                                               name=f"s2a{uname}", tag="s2a")
                            nc.vector.tensor_tensor(
                                out=sel2a[:, :ngrp, :]
                                    .rearrange("p c (h t) -> p c h t", t=2),
                                in0=iota2_sb
                                    .rearrange("p (h t) -> p h t", t=2)
                                    [:, None, :, :]
                                    .to_broadcast([P, ngrp, P, 2]),
                                in1=ld2a_sb
                                    .rearrange("p (c t) -> p c t", t=2)
                                    [:, c2c[0]:c2c[0] + ngrp, None, :]
                                    .to_broadcast([P, ngrp, P, 2]),
                                op=AOP.is_equal)
                            nquad = sum(1 for k, _ in batch if k == "q")
                            sel2b = spool.tile([P, 7, 2 * P], bf16,
                                               name=f"s2b{uname}", tag="s2b")
                            if nquad:
                                nc.vector.tensor_tensor(
                                    out=sel2b[:, :nquad, :]
                                    .rearrange("p c (h t) -> p c h t", t=2),
                                    in0=iota2_sb
                                    .rearrange("p (h t) -> p h t", t=2)
                                    [:, None, :, :]
                                    .to_broadcast([P, nquad, P, 2]),
                                    in1=ld2b_sb
                                    .rearrange("p (c t) -> p c t", t=2)
                                    [:, c2c[0]:c2c[0] + nquad, None, :]
                                    .to_broadcast([P, nquad, P, 2]),
                                    op=AOP.is_equal)
                            stg = stgpool.tile([P, 26 * 2 * P], bf16,
                                               name=f"st{uname}", tag="st")
                            gpos = 0
                            s1pos = 0
                            tpos = 0
                            ndr = 0
                            for gi, (kind, lts) in enumerate(batch):
                                ntl = len(lts)
                                c2pos = gpos + ntl
                                tpg = (2 * P) // F   # tiles per psum group
                                psb = None
                                for j, lt in enumerate(lts):
                                    if j % tpg == 0:
                                        psb = ps_sp.tile([P, 2 * P], f32,
                                                         name=f"ps{uname}",
                                                         tag="ps",
                                                         space="PSUM")
                                    k = j % tpg
                                    ps = psb[:, k * F:(k + 1) * F]
                                    nc.tensor.matmul(
                                        ps, lhsT=sel1[:, s1pos + j, :],
                                        rhs=g[:, gpos + j, :],
                                        start=True, stop=False)
                                    selc = sel2a if j < 2 else sel2b
                                    half = (j & 1) * P
                                    nc.tensor.matmul(
                                        ps,
                                        lhsT=selc[:, gi, half:half + P],
                                        rhs=g[:, c2pos, :],
                                        start=False, stop=True)
                                    if j % tpg == tpg - 1 or j == ntl - 1:
                                        # drain k+1 tiles, scaling by the
                                        # global dinv image (dinv^1 for
                                        # single passes; interleaved
                                        # dinv^1|dinv^2 for A|B halves of
                                        # paired passes)
                                        ntile = k + 1
                                        nls = ntile * F
                                        gt0 = r * RT + lts[j - k]
                                        dst3 = (stg[:, tpos * F:
                                                    tpos * F + nls]
                                                .rearrange("p (i f) -> p i f",
                                                           f=P))
                                        nhf = nls // P
                                        if F == P:
                                            im = (img128 if img128 is not None
                                                  else dg1_sb)
                                            img = im[:, gt0:gt0 + nhf, None]
                                        else:
                                            img = dgAB_sb[:, 2 * gt0:
                                                          2 * gt0 + nhf,
                                                          None]
                                        src3 = (psb[:, :nls]
                                                .rearrange("p (i f) -> p i f",
                                                           f=P))
                                        if F != P and ndr % 3 == 0:
                                            nc.vector.tensor_tensor(
                                                out=dst3, in0=src3,
                                                in1=img.to_broadcast(
                                                    [P, nhf, P]),
                                                op=AOP.mult)
                                        else:
                                            for ti in range(ntile):
                                                gt = gt0 + ti
                                                if F == P:
                                                    im0 = (img128 if img128
                                                           is not None
                                                           else dg1_sb)
                                                    nc.scalar.activation(
                                                        out=dst3[:, ti, :],
                                                        in_=src3[:, ti, :],
                                                        func=AF.Copy,
                                                        scale=im0[:,
                                                                  gt:gt + 1])
                                                else:
                                                    nc.scalar.activation(
                                                        out=dst3[:, 2 * ti,
                                                                 :],
                                                        in_=src3[:, 2 * ti,
                                                                 :],
                                                        func=AF.Copy,
                                                        scale=dg1_sb[
                                                            :, gt:gt + 1])
                                                    nc.scalar.activation(
                                                        out=dst3[:,
                                                                 2 * ti + 1,
                                                                 :],
                                                        in_=src3[:,
                                                                 2 * ti + 1,
                                                                 :],
                                                        func=AF.Copy,
                                                        scale=dg2_sb[
                                                            :, gt:gt + 1])
                                        ndr += 1
                                        tpos += ntile
                                gpos += ntl + 1
                                s1pos += ntl
                            # stage -> partial rows
                            rview = pbuf[rowbase:rowbase + nt * P, :]
                            nc.sync.dma_start(
                                rview.rearrange("(i p) f -> p i f", p=P),
                                stg[:, :nt * F]
                                .rearrange("p (i f) -> p i f", f=F))
                            rowbase += nt * P
                            slot0[0] += nidx
                            c1c[0] += nc1b
                            c2c[0] += ngrp
                    # RS for this phase
                    nc.gpsimd.collective_compute(
                        "ReduceScatter", AOP.add,
                        replica_groups=[list(range(NC))],
                        ins=[pbuf[:]], outs=[outs[ph]])

            def out_slices(buf):
                return (buf[0:6144, :], buf[6144:SHARD, :])

            # ------------------------------------------------ helpers
            def sweep(src, W, items):
                """Node-major dinv scaling: load src [SHARD, W] in 8-tile
                batches; for each (col0, dimg, dst_tensor, dcol0, f32out):
                dst rows = dimg-col(tile) * src[:, col0:col0+P]."""
                for t0 in range(0, RT, 4):
                    ntl = min(4, RT - t0)
                    ld = wpool.tile([P, 4, 2 * P], bf16, name="ssl",
                                    tag="ssl")
                    sv = src[t0 * P:(t0 + ntl) * P, :]
                    nc.sync.dma_start(
                        ld[:, :ntl, :W],
                        sv.rearrange("(i p) f -> p i f", p=P))
                    for col0, dimg, dst, dcol0, f32out in items:
                        dt_ = f32 if f32out else bf16
                        so = wpool.tile([P, 4, P], dt_, name="sso",
                                        tag=f"sso{1 if f32out else 0}")
                        if dimg is None:
                            nc.scalar.activation(
                                out=so[:, :ntl, :],
                                in_=ld[:, :ntl, col0:col0 + P],
                                func=AF.Copy)
                        else:
                            nc.vector.tensor_tensor(
                                out=so[:, :ntl, :],
                                in0=ld[:, :ntl, col0:col0 + P],
                                in1=dimg[:, t0:t0 + ntl, None]
                                    .to_broadcast([P, ntl, P]),
                                op=AOP.mult)
                        dv = dst[t0 * P:(t0 + ntl) * P, dcol0:dcol0 + P]
                        nc.sync.dma_start(
                            dv.rearrange("(i p) f -> p i f", p=P),
                            so[:, :ntl, :])

            st = {}

            def stat_sweep(buf, col0, keyS, keyQ):
                stS = st[keyS]; stQ = st[keyQ]
                for grp in range(NG):
                    n0 = grp * GSZ
                    w = min(GSZ, SHARD - n0)
                    par = grp % 2
                    tT = hpool.tile([P, GSZ], bf16, name="swT",
                                    tag="cpt" if par == 0 else "yT")
                    nc.sync.dma_start_transpose(
                        tT[:, :w], buf[n0:n0 + w, col0:col0 + P])
                    nc.vector.reduce_sum(out=stS[:, grp:grp + 1],
                                         in_=tT[:, :w],
                                         axis=mybir.AxisListType.X)
                    sq = wpool.tile([P, GSZ], f32, name="swq",
                                    tag="swq" if par == 0 else "sqt")
                    nc.vector.tensor_tensor(out=sq[:, :w], in0=tT[:, :w],
                                            in1=tT[:, :w], op=AOP.mult)
                    nc.vector.reduce_sum(out=stQ[:, grp:grp + 1],
                                         in_=sq[:, :w],
                                         axis=mybir.AxisListType.X)

            def bn_reduce_and_AB(l, keys):
                ar = wpool.tile([P, 8], f32, name=f"ar{l}", tag="ar")
                for b in range(3):
                    nc.vector.reduce_sum(out=ar[:, b:b + 1],
                                         in_=st[keys[2 * b]][:],
                                         axis=mybir.AxisListType.X)
                    nc.vector.reduce_sum(out=ar[:, 3 + b:4 + b],
                                         in_=st[keys[2 * b + 1]][:],
                                         axis=mybir.AxisListType.X)
                nc.sync.dma_start(arin[l][:], ar[:])
                nc.gpsimd.collective_compute(
                    "AllReduce", AOP.add, replica_groups=[list(range(NC))],
                    ins=[arin[l][:]], outs=[arout[l][:]])
                gg = wpool.tile([P, 8], f32, name=f"arg{l}", tag="ar")
                nc.sync.dma_start(gg[:], arout[l][:])
                A = stpool.tile([P, 3], f32, name=f"A{l}")
                B = stpool.tile([P, 3], f32, name=f"B{l}")
                mu = wpool.tile([P, 3], f32, name=f"mu{l}", tag="mu")
                va = wpool.tile([P, 3], f32, name=f"va{l}", tag="mu")
                nc.vector.tensor_scalar(out=mu[:], in0=gg[:, 0:3],
                                        scalar1=1.0 / N, scalar2=None,
                                        op0=AOP.mult)
                nc.vector.tensor_scalar(out=va[:], in0=gg[:, 3:6],
                                        scalar1=1.0 / N, scalar2=None,
                                        op0=AOP.mult)
                musq = wpool.tile([P, 3], f32, name=f"ms{l}", tag="mu")
                nc.vector.tensor_tensor(out=musq[:], in0=mu[:], in1=mu[:],
                                        op=AOP.mult)
                nc.vector.tensor_tensor(out=va[:], in0=va[:], in1=musq[:],
                                        op=AOP.subtract)
                ve = wpool.tile([P, 3], f32, name=f"ve{l}", tag="mu")
                nc.vector.tensor_scalar(out=ve[:], in0=va[:],
                                        scalar1=float(EPS), scalar2=None,
                                        op0=AOP.add)
                sq_ = wpool.tile([P, 3], f32, name=f"sv{l}", tag="mu")
                nc.scalar.activation(out=sq_[:], in_=ve[:], func=AF.Sqrt)
                rs = wpool.tile([P, 3], f32, name=f"rs{l}", tag="mu")
                nc.vector.reciprocal(out=rs[:], in_=sq_[:])
                muA = wpool.tile([P, 3], f32, name=f"ma{l}", tag="mu")
                for b in range(3):
                    nc.vector.tensor_tensor(out=A[:, b:b + 1],
                                            in0=rs[:, b:b + 1],
                                            in1=bng(l, b), op=AOP.mult)
                    nc.vector.tensor_tensor(out=muA[:, b:b + 1],
                                            in0=mu[:, b:b + 1],
                                            in1=A[:, b:b + 1], op=AOP.mult)
                    nc.vector.tensor_tensor(out=B[:, b:b + 1],
                                            in0=bnb(l, b),
                                            in1=muA[:, b:b + 1],
                                            op=AOP.subtract)
                return A, B

            # ================================================== pipeline
            if phase_sel == "t1":
                spmm_pass(xtbl_t[:], P, out_slices(u1buf), "u1")
                sweep(u1buf, P, [(0, None, out_t, 0, True)])

            if phase_sel == "full":
                for k in ("S0", "Q0", "S1", "Q1", "S2", "Q2"):
                    st[(1, k)] = stpool.tile([P, NG], f32, name=f"st1{k}")
                    st[(2, k)] = stpool.tile([P, NG], f32, name=f"st2{k}")

                # -------- L1 spmm chain

                # -------- L1 transform (hop-outer so hop-0 work runs
                # during the u-passes; hop-2 gates only its own tail)
                def l1_hop(hop):
                    for grp in range(NG):
                        n0 = grp * GSZ
                        w = min(GSZ, SHARD - n0)
                        yT = hpool.tile([P, GSZ], bf16, name="yT",
                                        tag="yT")
                        if hop == 0:
                            nc.sync.dma_start(yT[:, :w], xt_t[:, n0:n0 + w])
                        else:
                            ybuf = y1buf if hop == 1 else y2buf
                            nc.sync.dma_start_transpose(yT[:, :w],
                                                        ybuf[n0:n0 + w, :])
                        ps1 = ps_tf.tile([P, GSZ], f32, name="tf1", tag="tf",
                                         space="PSUM")
                        nc.tensor.matmul(ps1[:, :w],
                                         lhsT=w0_sb[:, hop * P:(hop + 1) * P],
                                         rhs=yT[:, :w], start=True,
                                         stop=False)
                        which = 2 if hop == 0 else hop - 1
                        nc.tensor.matmul(ps1[:, :w], lhsT=brow(0, hop),
                                         rhs=load_row(which, n0, w, "rw"),
                                         start=False, stop=True)
                        cp2 = hpool.tile([P, GSZ], bf16, name="cpt2",
                                         tag="cpt")
                        nc.scalar.activation(out=cp2[:, :w], in_=ps1[:, :w],
                                             func=AF.Copy)
                        nc.vector.reduce_sum(
                            out=st[(1, f"S{hop}")][:, grp:grp + 1],
                            in_=cp2[:, :w], axis=mybir.AxisListType.X)
                        sq2 = wpool.tile([P, GSZ], f32, name="sqt2",
                                         tag="sqt")
                        nc.vector.tensor_tensor(out=sq2[:, :w],
                                                in0=cp2[:, :w],
                                                in1=cp2[:, :w], op=AOP.mult)
                        nc.vector.reduce_sum(
                            out=st[(1, f"Q{hop}")][:, grp:grp + 1],
                            in_=sq2[:, :w], axis=mybir.AxisListType.X)
                        nc.sync.dma_start(raw1[hop][:, n0:n0 + w],
                                          cp2[:, :w])
                # interleave: hop-0 fills the u1-RS wait; hop-1 overlaps
                # u2's gathers; hop-2 runs after u2's RS.
                spmm_pass(xtbl_t[:], P, out_slices(tbl2), "u1",
                          img128=dg2_sb)
                l1_hop(0)
                sweep(tbl2, P, [(0, dlocI_sb, y1buf, 0, False)])
                l1_hop(1)
                spmm_pass(tbl2[:], P, out_slices(y2buf), "u2")
                l1_hop(2)
                A1, B1 = bn_reduce_and_AB(
                    0, [(1, k) for k in ("S0", "Q0", "S1", "Q1", "S2", "Q2")])

                # -------- transform L2 / L3
                def transform_layer(l, A, B, final):
                    # block sources: ("fm", buf [P,SHARD]) feature-major or
                    # ("nm", buf [SHARD,P]) node-major (transpose-load)
                    if l == 1:
                        srcs = [("fm", raw1[0], 0), ("fm", raw1[1], 0),
                                ("fm", raw1[2], 0)]
                    else:
                        srcs = [("fm", raw20, 0), ("nm", rab[0], 0),
                                ("nm", rb2[0], 0)]
                    for grp in range(NG):
                        n0 = grp * GSZ
                        w = min(GSZ, SHARD - n0)
                        nq = w // P
                        hbt = []
                        for b in range(3):
                            kind_b, src, c0 = srcs[b]
                            raw = hpool.tile([P, GSZ], bf16, name="raw",
                                             tag=f"raw{b}")
                            if kind_b == "fm":
                                nc.sync.dma_start(raw[:, :w],
                                                  src[:, n0:n0 + w])
                            else:
                                nc.sync.dma_start_transpose(
                                    raw[:, :w],
                                    src[n0:n0 + w, c0:c0 + P])
                            h = hpool.tile([P, GSZ], bf16, name="hh",
                                           tag=f"h{b}")
                            nc.scalar.activation(out=h[:, :w],
                                                 in_=raw[:, :w],
                                                 func=AF.Relu,
                                                 bias=B[:, b:b + 1],
                                                 scale=A[:, b:b + 1])
                            hbt.append(h)
                        for j in range(3):
                            ps = ps_tf.tile([P, GSZ], f32, name="tfj",
                                            tag="tf", space="PSUM")
                            for b in range(3):
                                nc.tensor.matmul(ps[:, :w],
                                                 lhsT=wblk(l, j, b),
                                                 rhs=hbt[b][:, :w],
                                                 start=(b == 0), stop=False)
                            nc.tensor.matmul(ps[:, :w], lhsT=brow(l, j),
                                             rhs=load_row(2, n0, w, "rw"),
                                             start=False, stop=True)
                            if j == 0 and not final:
                                cp = hpool.tile([P, GSZ], bf16, name="cpj",
                                                tag="cpt")
                                nc.scalar.activation(out=cp[:, :w],
                                                     in_=ps[:, :w],
                                                     func=AF.Copy)
                                nc.vector.reduce_sum(
                                    out=st[(2, "S0")][:, grp:grp + 1],
                                    in_=cp[:, :w], axis=mybir.AxisListType.X)
                                sq = wpool.tile([P, GSZ], f32, name="sqj",
                                                tag="sqt")
                                nc.vector.tensor_tensor(
                                    out=sq[:, :w], in0=cp[:, :w],
                                    in1=cp[:, :w], op=AOP.mult)
                                nc.vector.reduce_sum(
                                    out=st[(2, "Q0")][:, grp:grp + 1],
                                    in_=sq[:, :w], axis=mybir.AxisListType.X)
                                nc.sync.dma_start(raw20[:, n0:n0 + w],
                                                  cp[:, :w])
                            elif j == 0 and final:
                                cpf = wpool.tile([P, GSZ], f32, name="cpf",
                                                 tag="cpf")
                                nc.scalar.activation(out=cpf[:, :w],
                                                     in_=ps[:, :w],
                                                     func=AF.Copy)
                                stf = wpool.tile([P, 4, P], f32, name="of",
                                                 tag="of")
                                for q in range(nq):
                                    pst = ps_tp.tile([P, P], f32, name="ptf",
                                                     tag="tpf", space="PSUM",
                                                     bufs=1)
                                    nc.tensor.transpose(
                                        pst[:], cpf[:, q * P:(q + 1) * P],
                                        identf_sb)
                                    nc.scalar.activation(out=stf[:, q, :],
                                                         in_=pst[:],
                                                         func=AF.Copy)
                                dv = out_t[n0:n0 + w, 0:P]
                                nc.sync.dma_start(
                                    dv.rearrange("(i p) f -> p i f", p=P),
                                    stf[:, :nq, :])
                            else:
                                # t_j -> transpose -> dloc1-scale -> table
                                dst = tblpair[l - 1]
                                cpb = wpool.tile([P, GSZ], bf16,
                                                 name="cpb", tag="cpb")
                                nc.vector.tensor_copy(out=cpb[:, :w],
                                                      in_=ps[:, :w])
                                stgt = stgpool.tile([P, 4, P], bf16,
                                                    name="tstg",
                                                    tag=f"tstg{j}")
                                for q in range(nq):
                                    pst = ps_tp.tile([P, P], bf16,
                                                     name="ptb", tag="tp",
                                                     space="PSUM")
                                    nc.tensor.transpose(
                                        pst[:], cpb[:, q * P:(q + 1) * P],
                                        ident_sb)
                                    tt = n0 // P + q
                                    nc.scalar.activation(
                                        out=stgt[:, q, :], in_=pst[:],
                                        func=AF.Copy,
                                        scale=dloc1_sb[:, tt:tt + 1])
                                dv = dst[n0:n0 + w,
                                         (j - 1) * P:j * P]
                                nc.sync.dma_start(
                                    dv.rearrange("(i p) f -> p i f", p=P),
                                    stgt[:, :nq, :])

                # L2
                transform_layer(1, A1, B1, final=False)
                spmm_pass(tblpair[0][:], 2 * P, out_slices(rab[0]), "v1")
                sweep(rab[0], 2 * P, [(P, None, tblu[0], 0, False)])
                stat_sweep(rab[0], 0, (2, "S1"), (2, "Q1"))
                spmm_pass(tblu[0][:], P, out_slices(rb2[0]), "v2")
                stat_sweep(rb2[0], 0, (2, "S2"), (2, "Q2"))
                A2, B2 = bn_reduce_and_AB(
                    1, [(2, k) for k in ("S0", "Q0", "S1", "Q1", "S2", "Q2")])

                # L3
                transform_layer(2, A2, B2, final=True)
                spmm_pass(tblpair[1][:], 2 * P, out_slices(rab[1]), "w1")
                sweep(rab[1], 2 * P, [(P, None, tblu[1], 0, False)])
                spmm_pass(tblu[1][:], P, out_slices(rb2[1]), "w2")
                sweep(rab[1], 2 * P, [(0, None, out_t, P, True)])
                sweep(rb2[1], P, [(0, None, out_t, 2 * P, True)])

    if not for_sim:
        _split_excess_waits(nc)
        mybir.codegen_inst_isa_subclasses(nc)
    return nc


_CACHE = {}


def kernel(x, edge_index, W0, b0, W1, b1, W2, b2, bn_gamma, bn_beta):
    x = np.asarray(x, np.float32)
    edge_index = np.asarray(edge_index)
    cores = _host_prep(x, edge_index)

    W0 = np.asarray(W0, np.float32)
    W1 = np.asarray(W1, np.float32)
    W2 = np.asarray(W2, np.float32)
    b0 = np.asarray(b0, np.float32)
    b1 = np.asarray(b1, np.float32)
    b2 = np.asarray(b2, np.float32)
    bn_g = np.asarray(bn_gamma, np.float32)
    bn_b = np.asarray(bn_beta, np.float32)

    in_maps = []
    for c in range(NC):
        cb, cr, cf = _pack_consts(cores[c], W0, b0, W1, b1, W2, b2,
                                  bn_g, bn_b)
        in_maps.append(dict(
            xtbl=cores[c]["xtbl"], xt=cores[c]["xt"], idx=cores[c]["idx16"],
            cb=cb, cr=cr, cf=cf))

    phase = os.environ.get("KPHASE", "full")
    if phase not in _CACHE:
        _CACHE[phase] = _build(phase)
    nc = _CACHE[phase]
    res = run_bass_kernel_spmd(nc, in_maps, core_ids=list(range(NC)),
                               trace=bool(os.environ.get("KERNEL_TRACE")))
    global last_result
    last_result = res
    out = np.concatenate([r["out"] for r in res.results], axis=0)
    return out[:N].astype(np.float32)


last_result = None
